# revision 76
# baseline (speedup 1.0000x reference)
"""Trainium2 Bass kernel for nn_CAModel (neural cellular automaton step).

Strategy (pure data parallel, B=32 -> 4 images per core x 8 cores):
- Host pre-transposes to channel-major padded layout; device partition p =
  (img_local, half, channel) = 4*2*16 = 128.  All spatial shifts become
  free-dim offsets (row pitch 130, zero ring).
- Depthwise sobel conv as separable shifted adds on VectorE in bf16.
- fc0 as 3 accumulating K=32 matmuls per group (zero-padded weights per
  group parity), 4 partition strips run concurrently on the PE sub-arrays.
- relu PSUM->SBUF copy split between ScalarE and VectorE, bf16 out.
- fc1 as K=128 -> M=32 matmul pairs accumulating both group parities.
- residual + update mask + alive mask (3x3 maxpool in a strip layout,
  scatter/broadcast via SBUF-SBUF DMA) on VectorE.
"""

import dataclasses
import numpy as np
import ml_dtypes

import concourse.bass as bass
import concourse.tile as tile
from concourse import mybir, bass_utils
import bass_rust

F32 = mybir.dt.float32
BF16 = mybir.dt.bfloat16
FP8 = mybir.dt.float8e4
DR = mybir.MatmulPerfMode.DoubleRow
ALU = mybir.AluOpType
ACTF = mybir.ActivationFunctionType

N_CORES = 8
B, H, W, C = 32, 128, 128, 16
HID = 128
IMGS = B // N_CORES          # 4 images per core
GRP = IMGS * 2               # 8 (img, half) groups per core
PW = W + 2                   # padded row pitch 130
PR = H // 2 + 2              # padded rows per half 66
NPAD = PR * PW               # 8580
NPIX = (H // 2) * W          # 8192 interior pixels per group
CHUNK = 1024                 # pixels per MLP chunk (8 interior rows)
NCHUNK = NPIX // CHUNK       # 8
X2G = 128                    # guard elems around x2 free dim
RELU_PATTERN = (True, True, True, False)  # True -> ScalarE
# strip-in-half t -> partition block position: quarters {1-4},{5-8},{9-12},
# {13,14,15,0} unlock after chunks 2/4/6/7 respectively
TORD = [1, 2, 3, 4, 5, 6, 7, 8, 9, 10, 11, 12, 13, 14, 15, 0]
POS = {t: i for i, t in enumerate(TORD)}


def _split_multiwaits(nc):
    """walrus in this env only supports one sem-wait per instruction."""
    n = 0
    for f in nc.m.functions:
        for bb in f.blocks:
            out = []
            changed = False
            for inst in bb.instructions:
                si = inst.sync_info
                if si is not None and len(si.on_wait) > 1:
                    waits = list(si.on_wait)
                    for k, w in enumerate(waits[:-1]):
                        nop = mybir.InstNoOp(
                            name=f"{inst.name}_ws{k}",
                            sync_info=mybir.SyncInfo(on_wait=[w], on_update=[]),
                            bass_nofuse=True,
                            engine=inst.engine,
                        )
                        nc.register_instruction(nop, overwrite=True)
                        out.append(nop)
                        n += 1
                    inst.sync_info = mybir.SyncInfo(
                        on_wait=[waits[-1]], on_update=list(si.on_update)
                    )
                    changed = True
                out.append(inst)
            if changed:
                bb.instructions[:] = out
    return n


def _mk_ap(ap, offset, dims):
    return dataclasses.replace(ap, offset=offset, ap=[list(d) for d in dims])


def build_program():
    nc = bass.Bass()

    xb_d = nc.dram_tensor("xb", [128, NPAD], BF16, kind="ExternalInput").ap()
    x8_d = nc.dram_tensor("x8", [128, NPIX], FP8, kind="ExternalInput").ap()
    u16_d = nc.dram_tensor("u16", [128, NPIX], FP8, kind="ExternalInput").ap()
    astrip_d = nc.dram_tensor("astrip", [128, 780], F32, kind="ExternalInput").ap()
    # fp8 DoubleRow stationaries: pair A = (W0dx, W0id), pair B = (0, W0dy)
    w0_d = {}
    for pair in ("a", "b"):
        for gg in range(2):
            for j in range(4):
                w0_d[(pair, gg, j)] = nc.dram_tensor(
                    f"w0{pair}{gg}{j}", [128, 256], FP8, kind="ExternalInput"
                ).ap()
    w1_d = [
        nc.dram_tensor(f"w1dr{j}", [128, 256], FP8, kind="ExternalInput").ap()
        for j in range(4)
    ]
    sel_d = nc.dram_tensor("sel", [128, 2048], BF16, kind="ExternalInput").ap()
    out_d = nc.dram_tensor("out", [128, NPIX], BF16, kind="ExternalOutput").ap()

    with tile.TileContext(nc) as tc:
        with tc.tile_pool(name="persist", bufs=1) as pp:
            # --- persistent tiles ---
            xb = pp.tile([128, NPAD + 4], BF16, tag="xb")        # data at +2
            # dense fp8 feature buffer: [ydx | x | ydy], each [128, NPIX]
            feat8 = pp.tile([128, 3 * NPIX], FP8, tag="feat8")
            astrip = pp.tile([128, 780], F32, tag="astrip")
            a2strip = pp.tile([128, 780], BF16, tag="a2strip")
            selt = pp.tile([128, 2048], BF16, tag="selt")
            nc.sync.dma_start(out=selt[:, :], in_=sel_d)
            w0t = {k: pp.tile([128, 256], FP8, tag=f"w0{k[0]}{k[1]}{k[2]}", name=f"w0t{k[0]}{k[1]}{k[2]}") for k in w0_d}
            w1t = [
                pp.tile([128, 256], FP8, tag=f"w1dr{j}", name=f"w1t{j}")
                for j in range(4)
            ]

            # --- input DMAs ---
            for k in w0_d:
                nc.sync.dma_start(out=w0t[k][:, :], in_=w0_d[k])
            for j in range(4):
                nc.sync.dma_start(out=w1t[j][:, :], in_=w1_d[j])
            nc.sync.dma_start(out=feat8[:, NPIX : 2 * NPIX], in_=x8_d)
            CAST_BANDS = [(0, 18), (18, 34), (34, 50), (50, 66)]
            for lo, hi in CAST_BANDS:
                nc.sync.dma_start(
                    out=xb[:, 2 + lo * PW : 2 + hi * PW],
                    in_=xb_d[:, lo * PW : hi * PW],
                )
            nc.sync.dma_start(out=astrip[:, :], in_=astrip_d)

            # --- conv stages: stage s covers image rows [16s, 16s+16).
            # All temps stage-local (margin rows recomputed), so stages can
            # be emitted interleaved with the MLP chunks that consume them.
            P8 = 3 * NPIX        # feat8 partition pitch
            SROWS = 8            # image rows per conv stage
            PXX = (SROWS + 2) * PW      # xx2/t_b/th stage-tile pitch
            PTV = SROWS * PW + 4        # t_a/tv stage-tile pitch

            def emit_stage(s):
                pr0 = SROWS * s              # th/xx2 base padded row
                n_th = min(pr0 + SROWS + 2, 66) - pr0
                il = pr0 + 1                 # interior padded row range
                xx2s = pp.tile([128, PXX], BF16, tag="xx2", bufs=2)
                t_as = pp.tile([128, PTV], BF16, tag="ta", bufs=2)
                tvs = pp.tile([128, PTV], BF16, tag="tv", bufs=2)
                t_bs = pp.tile([128, PXX + 4], BF16, tag="tb", bufs=2)
                ths = pp.tile([128, PXX + 4], BF16, tag="th", bufs=2)
                nc.scalar.activation(
                    out=xx2s[:, 0 : n_th * PW],
                    in_=xb[:, 2 + pr0 * PW : 2 + (pr0 + n_th) * PW],
                    func=ACTF.Copy, scale=2.0,
                )
                nc.gpsimd.tensor_tensor(
                    out=t_bs[:, 2 : 2 + n_th * PW],
                    in0=xb[:, 1 + pr0 * PW : 1 + (pr0 + n_th) * PW],
                    in1=xb[:, 3 + pr0 * PW : 3 + (pr0 + n_th) * PW],
                    op=ALU.add,
                )
                nc.vector.tensor_tensor(
                    out=ths[:, 2 : 2 + n_th * PW],
                    in0=t_bs[:, 2 : 2 + n_th * PW],
                    in1=xx2s[:, 0 : n_th * PW],
                    op=ALU.add,
                )
                nc.vector.tensor_tensor(
                    out=t_as[:, 2 : 2 + SROWS * PW],
                    in0=xb[:, 2 + (il - 1) * PW : 2 + (il + SROWS - 1) * PW],
                    in1=xb[:, 2 + (il + 1) * PW : 2 + (il + SROWS + 1) * PW],
                    op=ALU.add,
                )
                nc.vector.tensor_tensor(
                    out=tvs[:, 2 : 2 + SROWS * PW],
                    in0=t_as[:, 2 : 2 + SROWS * PW],
                    in1=xx2s[:, 1 * PW : (SROWS + 1) * PW],
                    op=ALU.add,
                )
                # ydx -> feat8[0], ydy -> feat8[2], dense rows [8s,8s+8)
                nc.vector.tensor_tensor(
                    out=_mk_ap(feat8[:, :], SROWS * s * 128,
                               [[P8, 128], [128, SROWS], [1, 128]]),
                    in0=_mk_ap(tvs[:, :], 4, [[PTV, 128], [PW, SROWS], [1, 128]]),
                    in1=_mk_ap(tvs[:, :], 2, [[PTV, 128], [PW, SROWS], [1, 128]]),
                    op=ALU.subtract,
                )
                nc.vector.tensor_tensor(
                    out=_mk_ap(feat8[:, :], 2 * NPIX + SROWS * s * 128,
                               [[P8, 128], [128, SROWS], [1, 128]]),
                    in0=_mk_ap(ths[:, :], 2 * PW + 3,
                               [[PXX + 4, 128], [PW, SROWS], [1, 128]]),
                    in1=_mk_ap(ths[:, :], 3,
                               [[PXX + 4, 128], [PW, SROWS], [1, 128]]),
                    op=ALU.subtract,
                )
            emit_stage(0)
            emit_stage(1)

            # --- MLP + residual ---
            xbr = xb[:, 2 : 2 + NPAD].rearrange("p (r w) -> p r w", w=PW)
            xintr = xbr

            relu_i = 0
            lp_cm = tc.tile_pool(name="late", bufs=1)
            lp = lp_cm.__enter__()
            x2 = lp.tile([128, NPIX + 2 * X2G], BF16, tag="x2")  # data at +X2G
            nc.vector.memset(x2[:, 0:X2G], 0.0)
            nc.vector.memset(x2[:, X2G + NPIX : NPIX + 2 * X2G], 0.0)
            u16 = lp.tile([128, NPIX], FP8, tag="ul", name="u16")
            nc.sync.dma_start(out=u16[:, :], in_=u16_d)
            x2r = x2[:, X2G : X2G + NPIX].rearrange("p (r w) -> p r w", w=W)
            with (
                tc.tile_pool(name="mlp", bufs=1) as mp,
                tc.tile_pool(name="psum", bufs=1, space="PSUM") as psp,
            ):
                prepool = pp.tile([128, 512], F32, tag="prepool")

                # dummy matmul burst: >=3.4us of sustained PE activity lifts
                # the HAM clock gate to 2.4 GHz before the real MLP begins
                warm = psp.tile([128, 512], F32, tag="lps", bufs=2)
                for _w in range(32):
                    nc.tensor.matmul(
                        warm[:, 0:256],
                        w0t[("a", 0, 0)][:, 0:128],
                        w0t[("b", 0, 0)][:, :],
                        start=True, stop=True,
                    )

                def emit_prepool():
                    vm_e = pp.tile([128, 524], F32, tag="vm_e")
                    t1_e = pp.tile([128, 524], F32, tag="t1_e")
                    t2_e = pp.tile([128, 524], F32, tag="t2_e")
                    nc.vector.tensor_tensor(
                        out=t1_e[:, 0:520], in0=astrip[:, 0:520],
                        in1=astrip[:, 130:650], op=ALU.max,
                    )
                    nc.vector.tensor_tensor(
                        out=vm_e[:, 0:520], in0=t1_e[:, 0:520],
                        in1=astrip[:, 260:780], op=ALU.max,
                    )
                    nc.vector.tensor_tensor(
                        out=t2_e[:, 0:519], in0=vm_e[:, 0:519], in1=vm_e[:, 1:520],
                        op=ALU.max,
                    )
                    _vmr = vm_e[:, 0:520].rearrange("p (r w) -> p r w", w=130)
                    _t2r = t2_e[:, 0:520].rearrange("p (r w) -> p r w", w=130)
                    _ppr = prepool[:, :].rearrange("p (r w) -> p r w", w=128)
                    nc.vector.tensor_tensor(
                        out=_ppr[:, 0:4, :], in0=_t2r[:, 0:4, 0:128],
                        in1=_vmr[:, 0:4, 2:130], op=ALU.max,
                    )

                # --- per-quarter alive-mask machinery ---
                PITCH = NPIX + 2 * X2G
                x2ap = x2[:, :]
                a2ap = a2strip[:, :]
                nc.vector.memset(a2strip[:, :], 0.0)
                postpool = lp.tile([128, 512], F32, tag="postpool")
                pmin = lp.tile([128, 512], F32, tag="pmin")
                lifes = lp.tile([128, 512], BF16, tag="lifes")

                def scatter_t(t):
                    # halo rows for strip t, all 8 (img,half) groups
                    src = _mk_ap(
                        x2ap, 3 * PITCH + 512 * t,
                        [[16 * PITCH, 8], [128, 6], [1, 128]],
                    )
                    dst = _mk_ap(
                        a2ap, 780 * 8 * POS[t] + 1,
                        [[780, 8], [130, 6], [1, 128]],
                    )
                    nc.sync.dma_start(out=dst, in_=src)

                def pool_range(pl, ph):
                    sl = slice(pl, ph)
                    vm = lp.tile([128, 524], F32, tag="vm")
                    t1 = lp.tile([128, 524], F32, tag="t1")
                    t2 = lp.tile([128, 524], F32, tag="t2")
                    nc.vector.tensor_tensor(
                        out=t1[sl, 0:520], in0=a2strip[sl, 0:520],
                        in1=a2strip[sl, 130:650], op=ALU.max,
                    )
                    nc.vector.tensor_tensor(
                        out=vm[sl, 0:520], in0=t1[sl, 0:520],
                        in1=a2strip[sl, 260:780], op=ALU.max,
                    )
                    nc.vector.tensor_tensor(
                        out=t2[sl, 0:519], in0=vm[sl, 0:519], in1=vm[sl, 1:520],
                        op=ALU.max,
                    )
                    vmr = vm[sl, 0:520].rearrange("p (r w) -> p r w", w=130)
                    t2r = t2[sl, 0:520].rearrange("p (r w) -> p r w", w=130)
                    ppr = postpool[sl, :].rearrange("p (r w) -> p r w", w=128)
                    nc.vector.tensor_tensor(
                        out=ppr[:, 0:4, :], in0=t2r[:, 0:4, 0:128],
                        in1=vmr[:, 0:4, 2:130], op=ALU.max,
                    )
                    nc.vector.tensor_tensor(
                        out=pmin[sl, :], in0=prepool[sl, :],
                        in1=postpool[sl, :], op=ALU.min,
                    )
                    nc.vector.tensor_scalar(
                        out=lifes[sl, :], in0=pmin[sl, :], scalar1=0.1,
                        scalar2=None, op0=ALU.is_gt,
                    )

                def finish_t(t):
                    p0 = 32 * (POS[t] // 4)
                    lps = psp.tile([128, 512], F32, tag="lps", bufs=2)
                    nc.tensor.matmul(
                        lps[:, :],
                        selt[p0 : p0 + 32, 128 * t : 128 * t + 128],
                        lifes[p0 : p0 + 32, 0:512],
                        start=True, stop=True,
                        tile_position=(p0, 0),
                    )
                    ot = lp.tile([128, 512], BF16, tag="ot", bufs=2)
                    nc.vector.tensor_tensor(
                        out=ot[:, :],
                        in0=x2[:, X2G + 512 * t : X2G + 512 * t + 512],
                        in1=lps[:, :], op=ALU.mult,
                    )
                    eng = nc.sync if t % 2 == 0 else nc.scalar
                    eng.dma_start(
                        out=out_d[:, 512 * t : 512 * t + 512], in_=ot[:, :]
                    )

                for k in range(NCHUNK):
                    if k < NCHUNK - 2:
                        emit_stage(k + 2)
                    if k == 1:
                        emit_prepool()
                    if k == 3:
                        for t in (0, 1, 2, 3, 4):
                            scatter_t(t)
                        pool_range(0, 32)
                        for t in (1, 2, 3, 4):
                            finish_t(t)
                    if k == 5:
                        for t in (5, 6, 7, 8):
                            scatter_t(t)
                        pool_range(32, 64)
                        for t in (5, 6, 7, 8):
                            finish_t(t)
                    if k == 7:
                        for t in (9, 10, 11, 12):
                            scatter_t(t)
                        pool_range(64, 96)
                        for t in (9, 10, 11, 12):
                            finish_t(t)
                    r0 = 8 * k  # interior row base of chunk
                    P8 = 3 * NPIX
                    dxp = [
                        psp.tile([128, 512], F32, tag="dxp", bufs=2,
                                 name=f"dxp{k}_{_s}")
                        for _s in range(2)
                    ]
                    for j in range(4):
                        rh2 = mp.tile([128, 2048], FP8, tag="rh", bufs=4)
                        for gg in range(2):
                            hp = psp.tile([128, CHUNK], F32, tag="hp", bufs=2)
                            for sub in range(2):
                                rr = r0 + 4 * sub
                                hps = hp[:, sub * 512 : sub * 512 + 512]
                                # k-tile pairs: A = (ydx, x), B = (x*0, ydy)
                                rhsA = _mk_ap(
                                    feat8[:, :], rr * 128,
                                    [[P8, 128], [NPIX, 2], [1, 512]],
                                )
                                rhsB = _mk_ap(
                                    feat8[:, :], NPIX + rr * 128,
                                    [[P8, 128], [NPIX, 2], [1, 512]],
                                )
                                lhsA = _mk_ap(
                                    w0t[("a", gg, j)][:, :], 0,
                                    [[256, 128], [128, 2], [1, 128]],
                                )
                                lhsB = _mk_ap(
                                    w0t[("b", gg, j)][:, :], 0,
                                    [[256, 128], [128, 2], [1, 128]],
                                )
                                nc.tensor.matmul(
                                    hps, lhsA, rhsA,
                                    start=True, stop=False, perf_mode=DR,
                                )
                                nc.tensor.matmul(
                                    hps, lhsB, rhsB,
                                    start=False, stop=True, perf_mode=DR,
                                )
                            rhs = rh2[:, gg * 1024 : gg * 1024 + 1024]
                            if relu_i < 32 or RELU_PATTERN[relu_i % len(RELU_PATTERN)]:
                                nc.scalar.activation(
                                    out=rhs, in_=hp[:, :], func=ACTF.Relu
                                )
                            else:
                                nc.vector.tensor_scalar_max(rhs, hp[:, :], 0.0)
                            relu_i += 1
                        for sub in range(2):
                            rhs1 = _mk_ap(
                                rh2[:, :], sub * 512,
                                [[2048, 128], [1024, 2], [1, 512]],
                            )
                            lhs1 = _mk_ap(
                                w1t[j][:, :], 0, [[256, 128], [128, 2], [1, 128]],
                            )
                            nc.tensor.matmul(
                                dxp[sub][:, :],
                                lhs1, rhs1,
                                start=(j == 0), stop=(j == 3), perf_mode=DR,
                            )
                    st = lp.tile([128, 2048], BF16, tag="st", name=f"st{k}", bufs=2)
                    for sub in range(2):
                        nc.vector.tensor_tensor(
                            out=st[:, sub * 512 : sub * 512 + 512],
                            in0=dxp[sub][:, :],
                            in1=u16[:, k * CHUNK + sub * 512 :
                                    k * CHUNK + sub * 512 + 512],
                            op=ALU.mult,
                        )
                    str_ = st[:, 0:CHUNK].rearrange("p (r w) -> p r w", w=W)
                    nc.gpsimd.tensor_tensor(
                        out=x2r[:, r0 : r0 + 8, :],
                        in0=xintr[:, 1 + r0 : 9 + r0, 1:129],
                        in1=str_,
                        op=ALU.add,
                    )

                # --- last quarter: strips 13,14,15,0 (need final x2) ---
                for t in (13, 14, 15):
                    scatter_t(t)
                # cross-half halo fills: h0/t15 row5 <- half1 row 0;
                # h1/t0 row0 <- half0 row 63
                nc.sync.dma_start(
                    out=_mk_ap(a2ap, (8 * POS[15]) * 780 + 5 * 130 + 1,
                               [[780 * 2, 4], [1, 128]]),
                    in_=_mk_ap(x2ap, 19 * PITCH + X2G,
                               [[32 * PITCH, 4], [1, 128]]),
                )
                nc.sync.dma_start(
                    out=_mk_ap(a2ap, (8 * POS[0] + 1) * 780 + 1,
                               [[780 * 2, 4], [1, 128]]),
                    in_=_mk_ap(x2ap, 3 * PITCH + X2G + 63 * 128,
                               [[32 * PITCH, 4], [1, 128]]),
                )
                pool_range(96, 128)
                for t in (13, 14, 15, 0):
                    finish_t(t)

            lp_cm.__exit__(None, None, None)

    _split_multiwaits(nc)
    return nc


def host_prep(x, w0, w1, rand_mask):
    bf = ml_dtypes.bfloat16
    f8 = ml_dtypes.float8_e4m3fn
    S = 8.0
    xt = np.ascontiguousarray(x.transpose(0, 3, 1, 2))  # [B, C, H, W]

    xp = np.zeros((B, 2, C, PR, PW), bf)
    xp[:, 0, :, 1:66, 1:129] = xt[:, :, 0:65, :].astype(bf)
    xp[:, 1, :, 0:65, 1:129] = xt[:, :, 63:128, :].astype(bf)
    xp = xp.reshape(B, 2, C, NPAD)

    x8 = np.stack([xt[:, :, 0:64, :], xt[:, :, 64:128, :]], axis=1)  # [B,2,C,64,W]
    x8 = x8.astype(f8).reshape(B, 2, C, NPIX)

    # dxp comes out scaled by S*S (weights pre-scaled for fp8) -> fold 1/S^2
    u = (rand_mask[..., 0] <= 0.5).astype(np.float32).reshape(B, 2, 64, W) / (S * S)
    u16 = np.ascontiguousarray(
        np.broadcast_to(u[:, :, None], (B, 2, C, 64, W))
    ).astype(f8).reshape(B, 2, C, NPIX)

    apad = np.zeros((B, H + 2, PW), np.float32)
    apad[:, 1:129, 1:129] = x[..., 3]
    idx = 4 * np.arange(32)[:, None] + np.arange(6)[None, :]
    astr = apad[:, idx, :].reshape(B, 32, 780)  # [B, strip, 6*130]
    # partition layout (pos, i, h): p = 8*POS[t] + 2i + h  (strip s = 16h + t)
    astr = astr.reshape(B // IMGS, IMGS, 2, 16, 780).transpose(0, 3, 1, 2, 4)
    astr = astr[:, TORD]

    # fp8 weights, pre-scaled by S (the sobel /8 cancels S for dx/dy)
    blk_id = (w0[:, 0::3] * S).T.astype(f8)   # [16 c, 128 o]
    blk_dx = w0[:, 1::3].T.astype(f8)
    blk_dy = w0[:, 2::3].T.astype(f8)
    w0_arrs = {}
    for gg in range(2):
        for j in range(4):
            ta = np.zeros((128, 2, 128), f8)
            tb = np.zeros((128, 2, 128), f8)
            r = slice(32 * j + 16 * gg, 32 * j + 16 * gg + 16)
            ta[r, 0, :] = blk_dx     # k-tile 0 pairs with ydx
            ta[r, 1, :] = blk_id     # k-tile 1 pairs with x
            tb[r, 1, :] = blk_dy     # k-tile 0 is x * 0, tile 1 is ydy
            w0_arrs[("a", gg, j)] = ta.reshape(128, 256)
            w0_arrs[("b", gg, j)] = tb.reshape(128, 256)
    w1_arrs = []
    for j in range(4):
        t = np.zeros((128, 2, 128), f8)
        t[:, 0, 32 * j : 32 * j + 16] = (w1.T * S).astype(f8)
        t[:, 1, 32 * j + 16 : 32 * j + 32] = (w1.T * S).astype(f8)
        w1_arrs.append(t.reshape(128, 256))

    sel = np.zeros((128, 2048), bf)
    for t in range(16):
        for p in range(128):
            g = p // 16  # g = 2*i + h for p = 32i + 16h + c
            sel[8 * POS[t] + g, 128 * t + p] = 1.0

    in_maps = []
    for ci in range(N_CORES):
        sl = slice(IMGS * ci, IMGS * (ci + 1))
        m = {
            "xb": np.ascontiguousarray(xp[sl]).reshape(128, NPAD),
            "x8": np.ascontiguousarray(x8[sl]).reshape(128, NPIX),
            "u16": np.ascontiguousarray(u16[sl]).reshape(128, NPIX),
            "astrip": np.ascontiguousarray(astr[ci]).reshape(128, 780),
            "sel": sel,
        }
        for j in range(4):
            m[f"w1dr{j}"] = w1_arrs[j]
        for (pair, gg, j), arr in w0_arrs.items():
            m[f"w0{pair}{gg}{j}"] = arr
        in_maps.append(m)
    return in_maps


def host_post(results):
    out = np.empty((B, H, W, C), np.float32)
    for ci in range(N_CORES):
        o = results[ci]["out"].reshape(IMGS, 2, C, 64, W)
        out[IMGS * ci : IMGS * (ci + 1)] = o.transpose(0, 1, 3, 4, 2).reshape(
            IMGS, H, W, C
        )
    return out


_CACHE = {}


def kernel(x, w0, w1, rand_mask, _trace=False, _tmpdir=None):
    x = np.asarray(x, np.float32)
    w0 = np.asarray(w0, np.float32)
    w1 = np.asarray(w1, np.float32)
    rand_mask = np.asarray(rand_mask, np.float32)

    if "nc" not in _CACHE:
        _CACHE["nc"] = build_program()
    nc = _CACHE["nc"]

    in_maps = host_prep(x, w0, w1, rand_mask)
    res = bass_utils.run_bass_kernel_spmd(
        nc, in_maps, core_ids=list(range(N_CORES)), trace=_trace, tmpdir=_tmpdir
    )
    _CACHE["last_result"] = res
    return host_post(res.results)



# revision 79
# speedup vs baseline: 1.1400x; 1.1400x over previous
"""Trainium2 Bass kernel for nn_CAModel (neural cellular automaton step).

Strategy (pure data parallel, B=32 -> 4 images per core x 8 cores):
- Host pre-transposes to channel-major padded layout; device partition p =
  (img_local, half, channel) = 4*2*16 = 128.  All spatial shifts become
  free-dim offsets (row pitch 130, zero ring).
- Depthwise sobel conv as separable shifted adds on VectorE in bf16.
- fc0 as 3 accumulating K=32 matmuls per group (zero-padded weights per
  group parity), 4 partition strips run concurrently on the PE sub-arrays.
- relu PSUM->SBUF copy split between ScalarE and VectorE, bf16 out.
- fc1 as K=128 -> M=32 matmul pairs accumulating both group parities.
- residual + update mask + alive mask (3x3 maxpool in a strip layout,
  scatter/broadcast via SBUF-SBUF DMA) on VectorE.
"""

import dataclasses
import numpy as np
import ml_dtypes

import concourse.bass as bass
import concourse.tile as tile
from concourse import mybir, bass_utils
import bass_rust

F32 = mybir.dt.float32
BF16 = mybir.dt.bfloat16
FP8 = mybir.dt.float8e4
DR = mybir.MatmulPerfMode.DoubleRow
ALU = mybir.AluOpType
ACTF = mybir.ActivationFunctionType

N_CORES = 8
B, H, W, C = 32, 128, 128, 16
HID = 128
IMGS = B // N_CORES          # 4 images per core
GRP = IMGS * 2               # 8 (img, half) groups per core
PW = W + 2                   # padded row pitch 130
PR = H // 2 + 2              # padded rows per half 66
NPAD = PR * PW               # 8580
NPIX = (H // 2) * W          # 8192 interior pixels per group
CHUNK = 1024                 # pixels per MLP chunk (8 interior rows)
NCHUNK = NPIX // CHUNK       # 8
X2G = 128                    # guard elems around x2 free dim
RELU_PATTERN = (True, True, True, False)  # True -> ScalarE
# strip-in-half t -> partition block position: quarters {1-4},{5-8},{9-12},
# {13,14,15,0} unlock after chunks 2/4/6/7 respectively
TORD = [1, 2, 3, 4, 5, 6, 7, 8, 9, 10, 11, 12, 13, 14, 15, 0]
POS = {t: i for i, t in enumerate(TORD)}


def _split_multiwaits(nc):
    """walrus in this env only supports one sem-wait per instruction."""
    n = 0
    for f in nc.m.functions:
        for bb in f.blocks:
            out = []
            changed = False
            for inst in bb.instructions:
                si = inst.sync_info
                if si is not None and len(si.on_wait) > 1:
                    waits = list(si.on_wait)
                    for k, w in enumerate(waits[:-1]):
                        nop = mybir.InstNoOp(
                            name=f"{inst.name}_ws{k}",
                            sync_info=mybir.SyncInfo(on_wait=[w], on_update=[]),
                            bass_nofuse=True,
                            engine=inst.engine,
                        )
                        nc.register_instruction(nop, overwrite=True)
                        out.append(nop)
                        n += 1
                    inst.sync_info = mybir.SyncInfo(
                        on_wait=[waits[-1]], on_update=list(si.on_update)
                    )
                    changed = True
                out.append(inst)
            if changed:
                bb.instructions[:] = out
    return n


def _mk_ap(ap, offset, dims):
    return dataclasses.replace(ap, offset=offset, ap=[list(d) for d in dims])


def build_program():
    nc = bass.Bass()

    xb_d = nc.dram_tensor("xb", [128, NPAD], BF16, kind="ExternalInput").ap()
    x8_d = nc.dram_tensor("x8", [128, NPIX], FP8, kind="ExternalInput").ap()
    u16_d = nc.dram_tensor("u16", [128, NPIX], FP8, kind="ExternalInput").ap()
    astrip_d = nc.dram_tensor("astrip", [128, 780], F32, kind="ExternalInput").ap()
    # fp8 DoubleRow stationaries: pair A = (W0dx, W0id), pair B = (0, W0dy)
    w0_d = {}
    for pair in ("a", "b"):
        for gg in range(2):
            for j in range(4):
                w0_d[(pair, gg, j)] = nc.dram_tensor(
                    f"w0{pair}{gg}{j}", [128, 256], FP8, kind="ExternalInput"
                ).ap()
    w1_d = [
        nc.dram_tensor(f"w1dr{j}", [128, 256], FP8, kind="ExternalInput").ap()
        for j in range(4)
    ]
    sel_d = nc.dram_tensor("sel", [128, 2048], BF16, kind="ExternalInput").ap()
    out_d = nc.dram_tensor("out", [128, NPIX], BF16, kind="ExternalOutput").ap()

    with tile.TileContext(nc) as tc:
        with tc.tile_pool(name="persist", bufs=1) as pp:
            # --- persistent tiles ---
            xb = pp.tile([128, NPAD + 4], BF16, tag="xb")        # data at +2
            # dense fp8 feature buffer: [ydx | x | ydy], each [128, NPIX]
            feat8 = pp.tile([128, 3 * NPIX], FP8, tag="feat8")
            astrip = pp.tile([128, 780], F32, tag="astrip")
            a2strip = pp.tile([128, 780], BF16, tag="a2strip")
            selt = pp.tile([128, 2048], BF16, tag="selt")
            nc.sync.dma_start(out=selt[:, :], in_=sel_d)
            w0t = {k: pp.tile([128, 256], FP8, tag=f"w0{k[0]}{k[1]}{k[2]}", name=f"w0t{k[0]}{k[1]}{k[2]}") for k in w0_d}
            w1t = [
                pp.tile([128, 256], FP8, tag=f"w1dr{j}", name=f"w1t{j}")
                for j in range(4)
            ]

            # --- input DMAs ---
            for k in w0_d:
                nc.sync.dma_start(out=w0t[k][:, :], in_=w0_d[k])
            for j in range(4):
                nc.sync.dma_start(out=w1t[j][:, :], in_=w1_d[j])
            nc.sync.dma_start(out=feat8[:, NPIX : 2 * NPIX], in_=x8_d)
            CAST_BANDS = [(0, 18), (18, 34), (34, 50), (50, 66)]
            for lo, hi in CAST_BANDS:
                nc.sync.dma_start(
                    out=xb[:, 2 + lo * PW : 2 + hi * PW],
                    in_=xb_d[:, lo * PW : hi * PW],
                )
            nc.sync.dma_start(out=astrip[:, :], in_=astrip_d)

            # --- conv stages: stage s covers image rows [16s, 16s+16).
            # All temps stage-local (margin rows recomputed), so stages can
            # be emitted interleaved with the MLP chunks that consume them.
            P8 = 3 * NPIX        # feat8 partition pitch
            SROWS = 8            # image rows per conv stage
            PXX = (SROWS + 2) * PW      # xx2/t_b/th stage-tile pitch
            PTV = SROWS * PW + 4        # t_a/tv stage-tile pitch

            def emit_stage(s):
                pr0 = SROWS * s              # th/xx2 base padded row
                n_th = min(pr0 + SROWS + 2, 66) - pr0
                il = pr0 + 1                 # interior padded row range
                xx2s = pp.tile([128, PXX], BF16, tag="xx2", bufs=2)
                t_as = pp.tile([128, PTV], BF16, tag="ta", bufs=2)
                tvs = pp.tile([128, PTV], BF16, tag="tv", bufs=2)
                t_bs = pp.tile([128, PXX + 4], BF16, tag="tb", bufs=2)
                ths = pp.tile([128, PXX + 4], BF16, tag="th", bufs=2)
                nc.scalar.activation(
                    out=xx2s[:, 0 : n_th * PW],
                    in_=xb[:, 2 + pr0 * PW : 2 + (pr0 + n_th) * PW],
                    func=ACTF.Copy, scale=2.0,
                )
                nc.vector.tensor_tensor(
                    out=t_bs[:, 2 : 2 + n_th * PW],
                    in0=xb[:, 1 + pr0 * PW : 1 + (pr0 + n_th) * PW],
                    in1=xb[:, 3 + pr0 * PW : 3 + (pr0 + n_th) * PW],
                    op=ALU.add,
                )
                nc.vector.tensor_tensor(
                    out=ths[:, 2 : 2 + n_th * PW],
                    in0=t_bs[:, 2 : 2 + n_th * PW],
                    in1=xx2s[:, 0 : n_th * PW],
                    op=ALU.add,
                )
                nc.vector.tensor_tensor(
                    out=t_as[:, 2 : 2 + SROWS * PW],
                    in0=xb[:, 2 + (il - 1) * PW : 2 + (il + SROWS - 1) * PW],
                    in1=xb[:, 2 + (il + 1) * PW : 2 + (il + SROWS + 1) * PW],
                    op=ALU.add,
                )
                nc.vector.tensor_tensor(
                    out=tvs[:, 2 : 2 + SROWS * PW],
                    in0=t_as[:, 2 : 2 + SROWS * PW],
                    in1=xx2s[:, 1 * PW : (SROWS + 1) * PW],
                    op=ALU.add,
                )
                # ydx -> feat8[0], ydy -> feat8[2], dense rows [8s,8s+8)
                nc.vector.tensor_tensor(
                    out=_mk_ap(feat8[:, :], SROWS * s * 128,
                               [[P8, 128], [128, SROWS], [1, 128]]),
                    in0=_mk_ap(tvs[:, :], 4, [[PTV, 128], [PW, SROWS], [1, 128]]),
                    in1=_mk_ap(tvs[:, :], 2, [[PTV, 128], [PW, SROWS], [1, 128]]),
                    op=ALU.subtract,
                )
                nc.vector.tensor_tensor(
                    out=_mk_ap(feat8[:, :], 2 * NPIX + SROWS * s * 128,
                               [[P8, 128], [128, SROWS], [1, 128]]),
                    in0=_mk_ap(ths[:, :], 2 * PW + 3,
                               [[PXX + 4, 128], [PW, SROWS], [1, 128]]),
                    in1=_mk_ap(ths[:, :], 3,
                               [[PXX + 4, 128], [PW, SROWS], [1, 128]]),
                    op=ALU.subtract,
                )
            emit_stage(0)
            emit_stage(1)

            # --- MLP + residual ---
            xbr = xb[:, 2 : 2 + NPAD].rearrange("p (r w) -> p r w", w=PW)
            xintr = xbr

            relu_i = 0
            lp_cm = tc.tile_pool(name="late", bufs=1)
            lp = lp_cm.__enter__()
            x2 = lp.tile([128, NPIX + 2 * X2G], BF16, tag="x2")  # data at +X2G
            nc.vector.memset(x2[:, 0:X2G], 0.0)
            nc.vector.memset(x2[:, X2G + NPIX : NPIX + 2 * X2G], 0.0)
            u16 = lp.tile([128, NPIX], FP8, tag="ul", name="u16")
            nc.sync.dma_start(out=u16[:, :], in_=u16_d)
            x2r = x2[:, X2G : X2G + NPIX].rearrange("p (r w) -> p r w", w=W)
            with (
                tc.tile_pool(name="mlp", bufs=1) as mp,
                tc.tile_pool(name="psum", bufs=1, space="PSUM") as psp,
            ):
                prepool = pp.tile([128, 512], F32, tag="prepool")

                # dummy matmul burst: >=3.4us of sustained PE activity lifts
                # the HAM clock gate to 2.4 GHz before the real MLP begins
                warm = psp.tile([128, 512], F32, tag="lps", bufs=2)
                for _w in range(32):
                    nc.tensor.matmul(
                        warm[:, 0:256],
                        w0t[("a", 0, 0)][:, 0:128],
                        w0t[("b", 0, 0)][:, :],
                        start=True, stop=True,
                    )

                def emit_prepool():
                    vm_e = pp.tile([128, 524], F32, tag="vm_e")
                    t1_e = pp.tile([128, 524], F32, tag="t1_e")
                    t2_e = pp.tile([128, 524], F32, tag="t2_e")
                    nc.vector.tensor_tensor(
                        out=t1_e[:, 0:520], in0=astrip[:, 0:520],
                        in1=astrip[:, 130:650], op=ALU.max,
                    )
                    nc.vector.tensor_tensor(
                        out=vm_e[:, 0:520], in0=t1_e[:, 0:520],
                        in1=astrip[:, 260:780], op=ALU.max,
                    )
                    nc.vector.tensor_tensor(
                        out=t2_e[:, 0:519], in0=vm_e[:, 0:519], in1=vm_e[:, 1:520],
                        op=ALU.max,
                    )
                    _vmr = vm_e[:, 0:520].rearrange("p (r w) -> p r w", w=130)
                    _t2r = t2_e[:, 0:520].rearrange("p (r w) -> p r w", w=130)
                    _ppr = prepool[:, :].rearrange("p (r w) -> p r w", w=128)
                    nc.vector.tensor_tensor(
                        out=_ppr[:, 0:4, :], in0=_t2r[:, 0:4, 0:128],
                        in1=_vmr[:, 0:4, 2:130], op=ALU.max,
                    )

                # --- per-quarter alive-mask machinery ---
                PITCH = NPIX + 2 * X2G
                x2ap = x2[:, :]
                a2ap = a2strip[:, :]
                nc.vector.memset(a2strip[:, :], 0.0)
                postpool = lp.tile([128, 512], F32, tag="postpool")
                pmin = lp.tile([128, 512], F32, tag="pmin")
                lifes = lp.tile([128, 512], BF16, tag="lifes")

                def scatter_t(t):
                    # halo rows for strip t, all 8 (img,half) groups
                    src = _mk_ap(
                        x2ap, 3 * PITCH + 512 * t,
                        [[16 * PITCH, 8], [128, 6], [1, 128]],
                    )
                    dst = _mk_ap(
                        a2ap, 780 * 8 * POS[t] + 1,
                        [[780, 8], [130, 6], [1, 128]],
                    )
                    nc.sync.dma_start(out=dst, in_=src)

                def pool_range(pl, ph):
                    sl = slice(pl, ph)
                    vm = lp.tile([128, 524], F32, tag="vm")
                    t1 = lp.tile([128, 524], F32, tag="t1")
                    t2 = lp.tile([128, 524], F32, tag="t2")
                    nc.vector.tensor_tensor(
                        out=t1[sl, 0:520], in0=a2strip[sl, 0:520],
                        in1=a2strip[sl, 130:650], op=ALU.max,
                    )
                    nc.vector.tensor_tensor(
                        out=vm[sl, 0:520], in0=t1[sl, 0:520],
                        in1=a2strip[sl, 260:780], op=ALU.max,
                    )
                    nc.vector.tensor_tensor(
                        out=t2[sl, 0:519], in0=vm[sl, 0:519], in1=vm[sl, 1:520],
                        op=ALU.max,
                    )
                    vmr = vm[sl, 0:520].rearrange("p (r w) -> p r w", w=130)
                    t2r = t2[sl, 0:520].rearrange("p (r w) -> p r w", w=130)
                    ppr = postpool[sl, :].rearrange("p (r w) -> p r w", w=128)
                    nc.vector.tensor_tensor(
                        out=ppr[:, 0:4, :], in0=t2r[:, 0:4, 0:128],
                        in1=vmr[:, 0:4, 2:130], op=ALU.max,
                    )
                    nc.vector.tensor_tensor(
                        out=pmin[sl, :], in0=prepool[sl, :],
                        in1=postpool[sl, :], op=ALU.min,
                    )
                    nc.vector.tensor_scalar(
                        out=lifes[sl, :], in0=pmin[sl, :], scalar1=0.1,
                        scalar2=None, op0=ALU.is_gt,
                    )

                def finish_t(t):
                    p0 = 32 * (POS[t] // 4)
                    lps = psp.tile([128, 512], F32, tag="lps", bufs=2)
                    nc.tensor.matmul(
                        lps[:, :],
                        selt[p0 : p0 + 32, 128 * t : 128 * t + 128],
                        lifes[p0 : p0 + 32, 0:512],
                        start=True, stop=True,
                        tile_position=(p0, 0),
                    )
                    ot = lp.tile([128, 512], BF16, tag="ot", bufs=2)
                    nc.vector.tensor_tensor(
                        out=ot[:, :],
                        in0=x2[:, X2G + 512 * t : X2G + 512 * t + 512],
                        in1=lps[:, :], op=ALU.mult,
                    )
                    eng = nc.sync if t % 2 == 0 else nc.scalar
                    eng.dma_start(
                        out=out_d[:, 512 * t : 512 * t + 512], in_=ot[:, :]
                    )

                for k in range(NCHUNK):
                    if k < NCHUNK - 2:
                        emit_stage(k + 2)
                    if k == 1:
                        emit_prepool()
                    if k == 3:
                        for t in (0, 1, 2, 3, 4):
                            scatter_t(t)
                        pool_range(0, 32)
                        for t in (1, 2, 3, 4):
                            finish_t(t)
                    if k == 5:
                        for t in (5, 6, 7, 8):
                            scatter_t(t)
                        pool_range(32, 64)
                        for t in (5, 6, 7, 8):
                            finish_t(t)
                    if k == 7:
                        for t in (9, 10, 11, 12):
                            scatter_t(t)
                        pool_range(64, 96)
                        for t in (9, 10, 11, 12):
                            finish_t(t)
                    r0 = 8 * k  # interior row base of chunk
                    P8 = 3 * NPIX
                    dxp = [
                        psp.tile([128, 512], F32, tag="dxp", bufs=2,
                                 name=f"dxp{k}_{_s}")
                        for _s in range(2)
                    ]
                    for j in range(4):
                        rh2 = mp.tile([128, 2048], FP8, tag="rh", bufs=4)
                        for gg in range(2):
                            hp = psp.tile([128, CHUNK], F32, tag="hp", bufs=2)
                            for sub in range(2):
                                rr = r0 + 4 * sub
                                hps = hp[:, sub * 512 : sub * 512 + 512]
                                # k-tile pairs: A = (ydx, x), B = (x*0, ydy)
                                rhsA = _mk_ap(
                                    feat8[:, :], rr * 128,
                                    [[P8, 128], [NPIX, 2], [1, 512]],
                                )
                                rhsB = _mk_ap(
                                    feat8[:, :], NPIX + rr * 128,
                                    [[P8, 128], [NPIX, 2], [1, 512]],
                                )
                                lhsA = _mk_ap(
                                    w0t[("a", gg, j)][:, :], 0,
                                    [[256, 128], [128, 2], [1, 128]],
                                )
                                lhsB = _mk_ap(
                                    w0t[("b", gg, j)][:, :], 0,
                                    [[256, 128], [128, 2], [1, 128]],
                                )
                                nc.tensor.matmul(
                                    hps, lhsA, rhsA,
                                    start=True, stop=False, perf_mode=DR,
                                )
                                nc.tensor.matmul(
                                    hps, lhsB, rhsB,
                                    start=False, stop=True, perf_mode=DR,
                                )
                            rhs = rh2[:, gg * 1024 : gg * 1024 + 1024]
                            if relu_i < 16 or relu_i >= 48:
                                on_scalar = relu_i % 2 == 0
                            else:
                                on_scalar = relu_i % 4 != 3
                            if on_scalar:
                                nc.scalar.activation(
                                    out=rhs, in_=hp[:, :], func=ACTF.Relu
                                )
                            else:
                                nc.vector.tensor_scalar_max(rhs, hp[:, :], 0.0)
                            relu_i += 1
                        for sub in range(2):
                            rhs1 = _mk_ap(
                                rh2[:, :], sub * 512,
                                [[2048, 128], [1024, 2], [1, 512]],
                            )
                            lhs1 = _mk_ap(
                                w1t[j][:, :], 0, [[256, 128], [128, 2], [1, 128]],
                            )
                            nc.tensor.matmul(
                                dxp[sub][:, :],
                                lhs1, rhs1,
                                start=(j == 0), stop=(j == 3), perf_mode=DR,
                            )
                    st = lp.tile([128, 2048], BF16, tag="st", name=f"st{k}", bufs=2)
                    for sub in range(2):
                        nc.vector.tensor_tensor(
                            out=st[:, sub * 512 : sub * 512 + 512],
                            in0=dxp[sub][:, :],
                            in1=u16[:, k * CHUNK + sub * 512 :
                                    k * CHUNK + sub * 512 + 512],
                            op=ALU.mult,
                        )
                    str_ = st[:, 0:CHUNK].rearrange("p (r w) -> p r w", w=W)
                    nc.gpsimd.tensor_tensor(
                        out=x2r[:, r0 : r0 + 8, :],
                        in0=xintr[:, 1 + r0 : 9 + r0, 1:129],
                        in1=str_,
                        op=ALU.add,
                    )

                # --- last quarter: strips 13,14,15,0 (need final x2) ---
                for t in (13, 14, 15):
                    scatter_t(t)
                # cross-half halo fills: h0/t15 row5 <- half1 row 0;
                # h1/t0 row0 <- half0 row 63
                nc.sync.dma_start(
                    out=_mk_ap(a2ap, (8 * POS[15]) * 780 + 5 * 130 + 1,
                               [[780 * 2, 4], [1, 128]]),
                    in_=_mk_ap(x2ap, 19 * PITCH + X2G,
                               [[32 * PITCH, 4], [1, 128]]),
                )
                nc.sync.dma_start(
                    out=_mk_ap(a2ap, (8 * POS[0] + 1) * 780 + 1,
                               [[780 * 2, 4], [1, 128]]),
                    in_=_mk_ap(x2ap, 3 * PITCH + X2G + 63 * 128,
                               [[32 * PITCH, 4], [1, 128]]),
                )
                pool_range(96, 128)
                for t in (13, 14, 15, 0):
                    finish_t(t)

            lp_cm.__exit__(None, None, None)

    _split_multiwaits(nc)
    return nc


def host_prep(x, w0, w1, rand_mask):
    bf = ml_dtypes.bfloat16
    f8 = ml_dtypes.float8_e4m3fn
    S = 8.0
    xt = np.ascontiguousarray(x.transpose(0, 3, 1, 2))  # [B, C, H, W]

    xp = np.zeros((B, 2, C, PR, PW), bf)
    xp[:, 0, :, 1:66, 1:129] = xt[:, :, 0:65, :].astype(bf)
    xp[:, 1, :, 0:65, 1:129] = xt[:, :, 63:128, :].astype(bf)
    xp = xp.reshape(B, 2, C, NPAD)

    x8 = np.stack([xt[:, :, 0:64, :], xt[:, :, 64:128, :]], axis=1)  # [B,2,C,64,W]
    x8 = x8.astype(f8).reshape(B, 2, C, NPIX)

    # dxp comes out scaled by S*S (weights pre-scaled for fp8) -> fold 1/S^2
    u = (rand_mask[..., 0] <= 0.5).astype(np.float32).reshape(B, 2, 64, W) / (S * S)
    u16 = np.ascontiguousarray(
        np.broadcast_to(u[:, :, None], (B, 2, C, 64, W))
    ).astype(f8).reshape(B, 2, C, NPIX)

    apad = np.zeros((B, H + 2, PW), np.float32)
    apad[:, 1:129, 1:129] = x[..., 3]
    idx = 4 * np.arange(32)[:, None] + np.arange(6)[None, :]
    astr = apad[:, idx, :].reshape(B, 32, 780)  # [B, strip, 6*130]
    # partition layout (pos, i, h): p = 8*POS[t] + 2i + h  (strip s = 16h + t)
    astr = astr.reshape(B // IMGS, IMGS, 2, 16, 780).transpose(0, 3, 1, 2, 4)
    astr = astr[:, TORD]

    # fp8 weights, pre-scaled by S (the sobel /8 cancels S for dx/dy)
    blk_id = (w0[:, 0::3] * S).T.astype(f8)   # [16 c, 128 o]
    blk_dx = w0[:, 1::3].T.astype(f8)
    blk_dy = w0[:, 2::3].T.astype(f8)
    w0_arrs = {}
    for gg in range(2):
        for j in range(4):
            ta = np.zeros((128, 2, 128), f8)
            tb = np.zeros((128, 2, 128), f8)
            r = slice(32 * j + 16 * gg, 32 * j + 16 * gg + 16)
            ta[r, 0, :] = blk_dx     # k-tile 0 pairs with ydx
            ta[r, 1, :] = blk_id     # k-tile 1 pairs with x
            tb[r, 1, :] = blk_dy     # k-tile 0 is x * 0, tile 1 is ydy
            w0_arrs[("a", gg, j)] = ta.reshape(128, 256)
            w0_arrs[("b", gg, j)] = tb.reshape(128, 256)
    w1_arrs = []
    for j in range(4):
        t = np.zeros((128, 2, 128), f8)
        t[:, 0, 32 * j : 32 * j + 16] = (w1.T * S).astype(f8)
        t[:, 1, 32 * j + 16 : 32 * j + 32] = (w1.T * S).astype(f8)
        w1_arrs.append(t.reshape(128, 256))

    sel = np.zeros((128, 2048), bf)
    for t in range(16):
        for p in range(128):
            g = p // 16  # g = 2*i + h for p = 32i + 16h + c
            sel[8 * POS[t] + g, 128 * t + p] = 1.0

    in_maps = []
    for ci in range(N_CORES):
        sl = slice(IMGS * ci, IMGS * (ci + 1))
        m = {
            "xb": np.ascontiguousarray(xp[sl]).reshape(128, NPAD),
            "x8": np.ascontiguousarray(x8[sl]).reshape(128, NPIX),
            "u16": np.ascontiguousarray(u16[sl]).reshape(128, NPIX),
            "astrip": np.ascontiguousarray(astr[ci]).reshape(128, 780),
            "sel": sel,
        }
        for j in range(4):
            m[f"w1dr{j}"] = w1_arrs[j]
        for (pair, gg, j), arr in w0_arrs.items():
            m[f"w0{pair}{gg}{j}"] = arr
        in_maps.append(m)
    return in_maps


def host_post(results):
    out = np.empty((B, H, W, C), np.float32)
    for ci in range(N_CORES):
        o = results[ci]["out"].reshape(IMGS, 2, C, 64, W)
        out[IMGS * ci : IMGS * (ci + 1)] = o.transpose(0, 1, 3, 4, 2).reshape(
            IMGS, H, W, C
        )
    return out


_CACHE = {}


def kernel(x, w0, w1, rand_mask, _trace=False, _tmpdir=None):
    x = np.asarray(x, np.float32)
    w0 = np.asarray(w0, np.float32)
    w1 = np.asarray(w1, np.float32)
    rand_mask = np.asarray(rand_mask, np.float32)

    if "nc" not in _CACHE:
        _CACHE["nc"] = build_program()
    nc = _CACHE["nc"]

    in_maps = host_prep(x, w0, w1, rand_mask)
    res = bass_utils.run_bass_kernel_spmd(
        nc, in_maps, core_ids=list(range(N_CORES)), trace=_trace, tmpdir=_tmpdir
    )
    _CACHE["last_result"] = res
    return host_post(res.results)



# revision 82
# speedup vs baseline: 1.3119x; 1.1508x over previous
"""Trainium2 Bass kernel for nn_CAModel (neural cellular automaton step).

Strategy (pure data parallel, B=32 -> 4 images per core x 8 cores):
- Host pre-transposes to channel-major padded layout; device partition p =
  (img_local, half, channel) = 4*2*16 = 128.  All spatial shifts become
  free-dim offsets (row pitch 130, zero ring).
- Depthwise sobel conv as separable shifted adds on VectorE in bf16.
- fc0 as 3 accumulating K=32 matmuls per group (zero-padded weights per
  group parity), 4 partition strips run concurrently on the PE sub-arrays.
- relu PSUM->SBUF copy split between ScalarE and VectorE, bf16 out.
- fc1 as K=128 -> M=32 matmul pairs accumulating both group parities.
- residual + update mask + alive mask (3x3 maxpool in a strip layout,
  scatter/broadcast via SBUF-SBUF DMA) on VectorE.
"""

import dataclasses
import numpy as np
import ml_dtypes

import concourse.bass as bass
import concourse.tile as tile
from concourse import mybir, bass_utils
import bass_rust

F32 = mybir.dt.float32
BF16 = mybir.dt.bfloat16
FP8 = mybir.dt.float8e4
DR = mybir.MatmulPerfMode.DoubleRow
ALU = mybir.AluOpType
ACTF = mybir.ActivationFunctionType

N_CORES = 8
B, H, W, C = 32, 128, 128, 16
HID = 128
IMGS = B // N_CORES          # 4 images per core
GRP = IMGS * 2               # 8 (img, half) groups per core
PW = W + 2                   # padded row pitch 130
PR = H // 2 + 2              # padded rows per half 66
NPAD = PR * PW               # 8580
NPIX = (H // 2) * W          # 8192 interior pixels per group
CHUNK = 1024                 # pixels per MLP chunk (8 interior rows)
NCHUNK = NPIX // CHUNK       # 8
X2G = 128                    # guard elems around x2 free dim
RELU_PATTERN = (True, True, True, False)  # True -> ScalarE
# strip-in-half t -> partition block position: quarters {1-4},{5-8},{9-12},
# {13,14,15,0} unlock after chunks 2/4/6/7 respectively
TORD = [1, 2, 3, 4, 5, 6, 7, 8, 9, 10, 11, 12, 13, 14, 15, 0]
POS = {t: i for i, t in enumerate(TORD)}


def _split_multiwaits(nc):
    """walrus in this env only supports one sem-wait per instruction."""
    n = 0
    for f in nc.m.functions:
        for bb in f.blocks:
            out = []
            changed = False
            for inst in bb.instructions:
                si = inst.sync_info
                if si is not None and len(si.on_wait) > 1:
                    waits = list(si.on_wait)
                    for k, w in enumerate(waits[:-1]):
                        nop = mybir.InstNoOp(
                            name=f"{inst.name}_ws{k}",
                            sync_info=mybir.SyncInfo(on_wait=[w], on_update=[]),
                            bass_nofuse=True,
                            engine=inst.engine,
                        )
                        nc.register_instruction(nop, overwrite=True)
                        out.append(nop)
                        n += 1
                    inst.sync_info = mybir.SyncInfo(
                        on_wait=[waits[-1]], on_update=list(si.on_update)
                    )
                    changed = True
                out.append(inst)
            if changed:
                bb.instructions[:] = out
    return n


def _mk_ap(ap, offset, dims):
    return dataclasses.replace(ap, offset=offset, ap=[list(d) for d in dims])


def build_program():
    nc = bass.Bass()

    xb_d = nc.dram_tensor("xb", [128, NPAD], BF16, kind="ExternalInput").ap()
    x8_d = nc.dram_tensor("x8", [128, NPIX], FP8, kind="ExternalInput").ap()
    u16_d = nc.dram_tensor("u16", [128, NPIX], FP8, kind="ExternalInput").ap()
    astrip_d = nc.dram_tensor("astrip", [128, 780], F32, kind="ExternalInput").ap()
    # fp8 DoubleRow stationaries: pair A = (W0dx, W0id), pair B = (0, W0dy)
    w0_d = {}
    for pair in ("a", "b"):
        for gg in range(2):
            for j in range(4):
                w0_d[(pair, gg, j)] = nc.dram_tensor(
                    f"w0{pair}{gg}{j}", [128, 256], FP8, kind="ExternalInput"
                ).ap()
    w1_d = [
        nc.dram_tensor(f"w1dr{j}", [128, 256], FP8, kind="ExternalInput").ap()
        for j in range(4)
    ]
    sel_d = nc.dram_tensor("sel", [128, 2048], BF16, kind="ExternalInput").ap()
    out_d = nc.dram_tensor("out", [128, NPIX], BF16, kind="ExternalOutput").ap()

    with tile.TileContext(nc) as tc:
        with tc.tile_pool(name="persist", bufs=1) as pp:
            # --- persistent tiles ---
            xb = pp.tile([128, NPAD + 4], BF16, tag="xb")        # data at +2
            # dense fp8 feature buffer: [ydx | x | ydy], each [128, NPIX]
            feat8 = pp.tile([128, 3 * NPIX], FP8, tag="feat8")
            astrip = pp.tile([128, 780], F32, tag="astrip")
            a2strip = pp.tile([128, 780], BF16, tag="a2strip")
            selt = pp.tile([128, 2048], BF16, tag="selt")
            w0t = {k: pp.tile([128, 256], FP8, tag=f"w0{k[0]}{k[1]}{k[2]}", name=f"w0t{k[0]}{k[1]}{k[2]}") for k in w0_d}
            w1t = [
                pp.tile([128, 256], FP8, tag=f"w1dr{j}", name=f"w1t{j}")
                for j in range(4)
            ]

            # --- input DMAs: xb first (gates the conv), late-use tensors
            # (astrip/sel/u16) on the scalar HWDGE ring ---
            CAST_BANDS = [(0, 18), (18, 34), (34, 50), (50, 66)]
            for lo, hi in CAST_BANDS:
                nc.sync.dma_start(
                    out=xb[:, 2 + lo * PW : 2 + hi * PW],
                    in_=xb_d[:, lo * PW : hi * PW],
                )
            nc.sync.dma_start(out=feat8[:, NPIX : 2 * NPIX], in_=x8_d)
            for k in w0_d:
                nc.sync.dma_start(out=w0t[k][:, :], in_=w0_d[k])
            for j in range(4):
                nc.sync.dma_start(out=w1t[j][:, :], in_=w1_d[j])
            nc.scalar.dma_start(out=astrip[:, :], in_=astrip_d)
            nc.scalar.dma_start(out=selt[:, :], in_=sel_d)

            # --- conv stages: stage s covers image rows [16s, 16s+16).
            # All temps stage-local (margin rows recomputed), so stages can
            # be emitted interleaved with the MLP chunks that consume them.
            P8 = 3 * NPIX        # feat8 partition pitch
            SROWS = 8            # image rows per conv stage
            PXX = (SROWS + 2) * PW      # xx2/t_b/th stage-tile pitch
            PTV = SROWS * PW + 4        # t_a/tv stage-tile pitch

            def emit_stage(s):
                pr0 = SROWS * s              # th/xx2 base padded row
                n_th = min(pr0 + SROWS + 2, 66) - pr0
                il = pr0 + 1                 # interior padded row range
                xx2s = pp.tile([128, PXX], BF16, tag="xx2", bufs=2)
                t_as = pp.tile([128, PTV], BF16, tag="ta", bufs=2)
                tvs = pp.tile([128, PTV], BF16, tag="tv", bufs=2)
                t_bs = pp.tile([128, PXX + 4], BF16, tag="tb", bufs=2)
                ths = pp.tile([128, PXX + 4], BF16, tag="th", bufs=2)
                nc.scalar.activation(
                    out=xx2s[:, 0 : n_th * PW],
                    in_=xb[:, 2 + pr0 * PW : 2 + (pr0 + n_th) * PW],
                    func=ACTF.Copy, scale=2.0,
                )
                nc.vector.tensor_tensor(
                    out=t_bs[:, 2 : 2 + n_th * PW],
                    in0=xb[:, 1 + pr0 * PW : 1 + (pr0 + n_th) * PW],
                    in1=xb[:, 3 + pr0 * PW : 3 + (pr0 + n_th) * PW],
                    op=ALU.add,
                )
                nc.vector.tensor_tensor(
                    out=ths[:, 2 : 2 + n_th * PW],
                    in0=t_bs[:, 2 : 2 + n_th * PW],
                    in1=xx2s[:, 0 : n_th * PW],
                    op=ALU.add,
                )
                nc.vector.tensor_tensor(
                    out=t_as[:, 2 : 2 + SROWS * PW],
                    in0=xb[:, 2 + (il - 1) * PW : 2 + (il + SROWS - 1) * PW],
                    in1=xb[:, 2 + (il + 1) * PW : 2 + (il + SROWS + 1) * PW],
                    op=ALU.add,
                )
                nc.vector.tensor_tensor(
                    out=tvs[:, 2 : 2 + SROWS * PW],
                    in0=t_as[:, 2 : 2 + SROWS * PW],
                    in1=xx2s[:, 1 * PW : (SROWS + 1) * PW],
                    op=ALU.add,
                )
                # ydx -> feat8[0], ydy -> feat8[2], dense rows [8s,8s+8)
                nc.vector.tensor_tensor(
                    out=_mk_ap(feat8[:, :], SROWS * s * 128,
                               [[P8, 128], [128, SROWS], [1, 128]]),
                    in0=_mk_ap(tvs[:, :], 4, [[PTV, 128], [PW, SROWS], [1, 128]]),
                    in1=_mk_ap(tvs[:, :], 2, [[PTV, 128], [PW, SROWS], [1, 128]]),
                    op=ALU.subtract,
                )
                nc.vector.tensor_tensor(
                    out=_mk_ap(feat8[:, :], 2 * NPIX + SROWS * s * 128,
                               [[P8, 128], [128, SROWS], [1, 128]]),
                    in0=_mk_ap(ths[:, :], 2 * PW + 3,
                               [[PXX + 4, 128], [PW, SROWS], [1, 128]]),
                    in1=_mk_ap(ths[:, :], 3,
                               [[PXX + 4, 128], [PW, SROWS], [1, 128]]),
                    op=ALU.subtract,
                )
            emit_stage(0)
            emit_stage(1)

            # --- MLP + residual ---
            xbr = xb[:, 2 : 2 + NPAD].rearrange("p (r w) -> p r w", w=PW)
            xintr = xbr

            relu_i = 0
            lp_cm = tc.tile_pool(name="late", bufs=1)
            lp = lp_cm.__enter__()
            x2 = lp.tile([128, NPIX + 2 * X2G], BF16, tag="x2")  # data at +X2G
            nc.vector.memset(x2[:, 0:X2G], 0.0)
            nc.vector.memset(x2[:, X2G + NPIX : NPIX + 2 * X2G], 0.0)
            u16 = lp.tile([128, NPIX], FP8, tag="ul", name="u16")
            nc.scalar.dma_start(out=u16[:, :], in_=u16_d)
            x2r = x2[:, X2G : X2G + NPIX].rearrange("p (r w) -> p r w", w=W)
            with (
                tc.tile_pool(name="mlp", bufs=1) as mp,
                tc.tile_pool(name="psum", bufs=1, space="PSUM") as psp,
            ):
                prepool = pp.tile([128, 512], F32, tag="prepool")

                # dummy matmul burst: >=3.4us of sustained PE activity lifts
                # the HAM clock gate to 2.4 GHz before the real MLP begins
                warm = psp.tile([128, 512], F32, tag="lps", bufs=2)
                for _w in range(32):
                    nc.tensor.matmul(
                        warm[:, 0:256],
                        w0t[("a", 0, 0)][:, 0:128],
                        w0t[("b", 0, 0)][:, :],
                        start=True, stop=True,
                    )

                def emit_prepool():
                    vm_e = pp.tile([128, 524], F32, tag="vm_e")
                    t1_e = pp.tile([128, 524], F32, tag="t1_e")
                    t2_e = pp.tile([128, 524], F32, tag="t2_e")
                    nc.vector.tensor_tensor(
                        out=t1_e[:, 0:520], in0=astrip[:, 0:520],
                        in1=astrip[:, 130:650], op=ALU.max,
                    )
                    nc.vector.tensor_tensor(
                        out=vm_e[:, 0:520], in0=t1_e[:, 0:520],
                        in1=astrip[:, 260:780], op=ALU.max,
                    )
                    nc.vector.tensor_tensor(
                        out=t2_e[:, 0:519], in0=vm_e[:, 0:519], in1=vm_e[:, 1:520],
                        op=ALU.max,
                    )
                    _vmr = vm_e[:, 0:520].rearrange("p (r w) -> p r w", w=130)
                    _t2r = t2_e[:, 0:520].rearrange("p (r w) -> p r w", w=130)
                    _ppr = prepool[:, :].rearrange("p (r w) -> p r w", w=128)
                    nc.vector.tensor_tensor(
                        out=_ppr[:, 0:4, :], in0=_t2r[:, 0:4, 0:128],
                        in1=_vmr[:, 0:4, 2:130], op=ALU.max,
                    )

                # --- per-quarter alive-mask machinery ---
                PITCH = NPIX + 2 * X2G
                x2ap = x2[:, :]
                a2ap = a2strip[:, :]
                nc.vector.memset(a2strip[:, :], 0.0)
                postpool = lp.tile([128, 512], F32, tag="postpool")
                pmin = lp.tile([128, 512], F32, tag="pmin")
                lifes = lp.tile([128, 512], BF16, tag="lifes")

                def scatter_t(t):
                    # halo rows for strip t, all 8 (img,half) groups
                    src = _mk_ap(
                        x2ap, 3 * PITCH + 512 * t,
                        [[16 * PITCH, 8], [128, 6], [1, 128]],
                    )
                    dst = _mk_ap(
                        a2ap, 780 * 8 * POS[t] + 1,
                        [[780, 8], [130, 6], [1, 128]],
                    )
                    nc.sync.dma_start(out=dst, in_=src)

                def pool_range(pl, ph):
                    sl = slice(pl, ph)
                    vm = lp.tile([128, 524], F32, tag="vm")
                    t1 = lp.tile([128, 524], F32, tag="t1")
                    t2 = lp.tile([128, 524], F32, tag="t2")
                    nc.vector.tensor_tensor(
                        out=t1[sl, 0:520], in0=a2strip[sl, 0:520],
                        in1=a2strip[sl, 130:650], op=ALU.max,
                    )
                    nc.vector.tensor_tensor(
                        out=vm[sl, 0:520], in0=t1[sl, 0:520],
                        in1=a2strip[sl, 260:780], op=ALU.max,
                    )
                    nc.vector.tensor_tensor(
                        out=t2[sl, 0:519], in0=vm[sl, 0:519], in1=vm[sl, 1:520],
                        op=ALU.max,
                    )
                    vmr = vm[sl, 0:520].rearrange("p (r w) -> p r w", w=130)
                    t2r = t2[sl, 0:520].rearrange("p (r w) -> p r w", w=130)
                    ppr = postpool[sl, :].rearrange("p (r w) -> p r w", w=128)
                    nc.vector.tensor_tensor(
                        out=ppr[:, 0:4, :], in0=t2r[:, 0:4, 0:128],
                        in1=vmr[:, 0:4, 2:130], op=ALU.max,
                    )
                    nc.vector.tensor_tensor(
                        out=pmin[sl, :], in0=prepool[sl, :],
                        in1=postpool[sl, :], op=ALU.min,
                    )
                    nc.vector.tensor_scalar(
                        out=lifes[sl, :], in0=pmin[sl, :], scalar1=0.1,
                        scalar2=None, op0=ALU.is_gt,
                    )

                def finish_t(t):
                    p0 = 32 * (POS[t] // 4)
                    lps = psp.tile([128, 512], F32, tag="lps", bufs=2)
                    nc.tensor.matmul(
                        lps[:, :],
                        selt[p0 : p0 + 32, 128 * t : 128 * t + 128],
                        lifes[p0 : p0 + 32, 0:512],
                        start=True, stop=True,
                        tile_position=(p0, 0),
                    )
                    ot = lp.tile([128, 512], BF16, tag="ot", bufs=2)
                    nc.vector.tensor_tensor(
                        out=ot[:, :],
                        in0=x2[:, X2G + 512 * t : X2G + 512 * t + 512],
                        in1=lps[:, :], op=ALU.mult,
                    )
                    eng = nc.sync if t % 2 == 0 else nc.scalar
                    eng.dma_start(
                        out=out_d[:, 512 * t : 512 * t + 512], in_=ot[:, :]
                    )

                for k in range(NCHUNK):
                    if k < NCHUNK - 2:
                        emit_stage(k + 2)
                    if k == 1:
                        emit_prepool()
                    if k == 3:
                        for t in (0, 1, 2, 3, 4):
                            scatter_t(t)
                        pool_range(0, 32)
                        for t in (1, 2, 3, 4):
                            finish_t(t)
                    if k == 5:
                        for t in (5, 6, 7, 8):
                            scatter_t(t)
                        pool_range(32, 64)
                        for t in (5, 6, 7, 8):
                            finish_t(t)
                    if k == 7:
                        for t in (9, 10, 11, 12):
                            scatter_t(t)
                        pool_range(64, 96)
                        for t in (9, 10, 11, 12):
                            finish_t(t)
                    r0 = 8 * k  # interior row base of chunk
                    P8 = 3 * NPIX
                    dxp = [
                        psp.tile([128, 512], F32, tag="dxp", bufs=2,
                                 name=f"dxp{k}_{_s}")
                        for _s in range(2)
                    ]
                    for j in range(4):
                        rh2 = mp.tile([128, 2048], FP8, tag="rh", bufs=4)
                        for gg in range(2):
                            hp = psp.tile([128, CHUNK], F32, tag="hp", bufs=2)
                            for sub in range(2):
                                rr = r0 + 4 * sub
                                hps = hp[:, sub * 512 : sub * 512 + 512]
                                # k-tile pairs: A = (ydx, x), B = (x*0, ydy)
                                rhsA = _mk_ap(
                                    feat8[:, :], rr * 128,
                                    [[P8, 128], [NPIX, 2], [1, 512]],
                                )
                                rhsB = _mk_ap(
                                    feat8[:, :], NPIX + rr * 128,
                                    [[P8, 128], [NPIX, 2], [1, 512]],
                                )
                                lhsA = _mk_ap(
                                    w0t[("a", gg, j)][:, :], 0,
                                    [[256, 128], [128, 2], [1, 128]],
                                )
                                lhsB = _mk_ap(
                                    w0t[("b", gg, j)][:, :], 0,
                                    [[256, 128], [128, 2], [1, 128]],
                                )
                                nc.tensor.matmul(
                                    hps, lhsA, rhsA,
                                    start=True, stop=False, perf_mode=DR,
                                )
                                nc.tensor.matmul(
                                    hps, lhsB, rhsB,
                                    start=False, stop=True, perf_mode=DR,
                                )
                            rhs = rh2[:, gg * 1024 : gg * 1024 + 1024]
                            if relu_i < 16 or RELU_PATTERN[relu_i % len(RELU_PATTERN)]:
                                nc.scalar.activation(
                                    out=rhs, in_=hp[:, :], func=ACTF.Relu
                                )
                            else:
                                nc.vector.tensor_scalar_max(rhs, hp[:, :], 0.0)
                            relu_i += 1
                        for sub in range(2):
                            rhs1 = _mk_ap(
                                rh2[:, :], sub * 512,
                                [[2048, 128], [1024, 2], [1, 512]],
                            )
                            lhs1 = _mk_ap(
                                w1t[j][:, :], 0, [[256, 128], [128, 2], [1, 128]],
                            )
                            nc.tensor.matmul(
                                dxp[sub][:, :],
                                lhs1, rhs1,
                                start=(j == 0), stop=(j == 3), perf_mode=DR,
                            )
                    st = lp.tile([128, 2048], BF16, tag="st", name=f"st{k}", bufs=2)
                    for sub in range(2):
                        nc.vector.tensor_tensor(
                            out=st[:, sub * 512 : sub * 512 + 512],
                            in0=dxp[sub][:, :],
                            in1=u16[:, k * CHUNK + sub * 512 :
                                    k * CHUNK + sub * 512 + 512],
                            op=ALU.mult,
                        )
                    str_ = st[:, 0:CHUNK].rearrange("p (r w) -> p r w", w=W)
                    nc.gpsimd.tensor_tensor(
                        out=x2r[:, r0 : r0 + 8, :],
                        in0=xintr[:, 1 + r0 : 9 + r0, 1:129],
                        in1=str_,
                        op=ALU.add,
                    )

                # --- last quarter: strips 13,14,15,0 (need final x2) ---
                for t in (13, 14, 15):
                    scatter_t(t)
                # cross-half halo fills: h0/t15 row5 <- half1 row 0;
                # h1/t0 row0 <- half0 row 63
                nc.sync.dma_start(
                    out=_mk_ap(a2ap, (8 * POS[15]) * 780 + 5 * 130 + 1,
                               [[780 * 2, 4], [1, 128]]),
                    in_=_mk_ap(x2ap, 19 * PITCH + X2G,
                               [[32 * PITCH, 4], [1, 128]]),
                )
                nc.sync.dma_start(
                    out=_mk_ap(a2ap, (8 * POS[0] + 1) * 780 + 1,
                               [[780 * 2, 4], [1, 128]]),
                    in_=_mk_ap(x2ap, 3 * PITCH + X2G + 63 * 128,
                               [[32 * PITCH, 4], [1, 128]]),
                )
                pool_range(96, 128)
                for t in (13, 14, 15, 0):
                    finish_t(t)

            lp_cm.__exit__(None, None, None)

    _split_multiwaits(nc)
    return nc


def host_prep(x, w0, w1, rand_mask):
    bf = ml_dtypes.bfloat16
    f8 = ml_dtypes.float8_e4m3fn
    S = 8.0
    xt = np.ascontiguousarray(x.transpose(0, 3, 1, 2))  # [B, C, H, W]

    xp = np.zeros((B, 2, C, PR, PW), bf)
    xp[:, 0, :, 1:66, 1:129] = xt[:, :, 0:65, :].astype(bf)
    xp[:, 1, :, 0:65, 1:129] = xt[:, :, 63:128, :].astype(bf)
    xp = xp.reshape(B, 2, C, NPAD)

    x8 = np.stack([xt[:, :, 0:64, :], xt[:, :, 64:128, :]], axis=1)  # [B,2,C,64,W]
    x8 = x8.astype(f8).reshape(B, 2, C, NPIX)

    # dxp comes out scaled by S*S (weights pre-scaled for fp8) -> fold 1/S^2
    u = (rand_mask[..., 0] <= 0.5).astype(np.float32).reshape(B, 2, 64, W) / (S * S)
    u16 = np.ascontiguousarray(
        np.broadcast_to(u[:, :, None], (B, 2, C, 64, W))
    ).astype(f8).reshape(B, 2, C, NPIX)

    apad = np.zeros((B, H + 2, PW), np.float32)
    apad[:, 1:129, 1:129] = x[..., 3]
    idx = 4 * np.arange(32)[:, None] + np.arange(6)[None, :]
    astr = apad[:, idx, :].reshape(B, 32, 780)  # [B, strip, 6*130]
    # partition layout (pos, i, h): p = 8*POS[t] + 2i + h  (strip s = 16h + t)
    astr = astr.reshape(B // IMGS, IMGS, 2, 16, 780).transpose(0, 3, 1, 2, 4)
    astr = astr[:, TORD]

    # fp8 weights, pre-scaled by S (the sobel /8 cancels S for dx/dy)
    blk_id = (w0[:, 0::3] * S).T.astype(f8)   # [16 c, 128 o]
    blk_dx = w0[:, 1::3].T.astype(f8)
    blk_dy = w0[:, 2::3].T.astype(f8)
    w0_arrs = {}
    for gg in range(2):
        for j in range(4):
            ta = np.zeros((128, 2, 128), f8)
            tb = np.zeros((128, 2, 128), f8)
            r = slice(32 * j + 16 * gg, 32 * j + 16 * gg + 16)
            ta[r, 0, :] = blk_dx     # k-tile 0 pairs with ydx
            ta[r, 1, :] = blk_id     # k-tile 1 pairs with x
            tb[r, 1, :] = blk_dy     # k-tile 0 is x * 0, tile 1 is ydy
            w0_arrs[("a", gg, j)] = ta.reshape(128, 256)
            w0_arrs[("b", gg, j)] = tb.reshape(128, 256)
    w1_arrs = []
    for j in range(4):
        t = np.zeros((128, 2, 128), f8)
        t[:, 0, 32 * j : 32 * j + 16] = (w1.T * S).astype(f8)
        t[:, 1, 32 * j + 16 : 32 * j + 32] = (w1.T * S).astype(f8)
        w1_arrs.append(t.reshape(128, 256))

    sel = np.zeros((128, 2048), bf)
    for t in range(16):
        for p in range(128):
            g = p // 16  # g = 2*i + h for p = 32i + 16h + c
            sel[8 * POS[t] + g, 128 * t + p] = 1.0

    in_maps = []
    for ci in range(N_CORES):
        sl = slice(IMGS * ci, IMGS * (ci + 1))
        m = {
            "xb": np.ascontiguousarray(xp[sl]).reshape(128, NPAD),
            "x8": np.ascontiguousarray(x8[sl]).reshape(128, NPIX),
            "u16": np.ascontiguousarray(u16[sl]).reshape(128, NPIX),
            "astrip": np.ascontiguousarray(astr[ci]).reshape(128, 780),
            "sel": sel,
        }
        for j in range(4):
            m[f"w1dr{j}"] = w1_arrs[j]
        for (pair, gg, j), arr in w0_arrs.items():
            m[f"w0{pair}{gg}{j}"] = arr
        in_maps.append(m)
    return in_maps


def host_post(results):
    out = np.empty((B, H, W, C), np.float32)
    for ci in range(N_CORES):
        o = results[ci]["out"].reshape(IMGS, 2, C, 64, W)
        out[IMGS * ci : IMGS * (ci + 1)] = o.transpose(0, 1, 3, 4, 2).reshape(
            IMGS, H, W, C
        )
    return out


_CACHE = {}


def kernel(x, w0, w1, rand_mask, _trace=False, _tmpdir=None):
    x = np.asarray(x, np.float32)
    w0 = np.asarray(w0, np.float32)
    w1 = np.asarray(w1, np.float32)
    rand_mask = np.asarray(rand_mask, np.float32)

    if "nc" not in _CACHE:
        _CACHE["nc"] = build_program()
    nc = _CACHE["nc"]

    in_maps = host_prep(x, w0, w1, rand_mask)
    res = bass_utils.run_bass_kernel_spmd(
        nc, in_maps, core_ids=list(range(N_CORES)), trace=_trace, tmpdir=_tmpdir
    )
    _CACHE["last_result"] = res
    return host_post(res.results)



# revision 83
# speedup vs baseline: 1.3478x; 1.0274x over previous
"""Trainium2 Bass kernel for nn_CAModel (neural cellular automaton step).

Strategy (pure data parallel, B=32 -> 4 images per core x 8 cores):
- Host pre-transposes to channel-major padded layout; device partition p =
  (img_local, half, channel) = 4*2*16 = 128.  All spatial shifts become
  free-dim offsets (row pitch 130, zero ring).
- Depthwise sobel conv as separable shifted adds on VectorE in bf16.
- fc0 as 3 accumulating K=32 matmuls per group (zero-padded weights per
  group parity), 4 partition strips run concurrently on the PE sub-arrays.
- relu PSUM->SBUF copy split between ScalarE and VectorE, bf16 out.
- fc1 as K=128 -> M=32 matmul pairs accumulating both group parities.
- residual + update mask + alive mask (3x3 maxpool in a strip layout,
  scatter/broadcast via SBUF-SBUF DMA) on VectorE.
"""

import dataclasses
import numpy as np
import ml_dtypes

import concourse.bass as bass
import concourse.tile as tile
from concourse import mybir, bass_utils
import bass_rust

F32 = mybir.dt.float32
BF16 = mybir.dt.bfloat16
FP8 = mybir.dt.float8e4
DR = mybir.MatmulPerfMode.DoubleRow
ALU = mybir.AluOpType
ACTF = mybir.ActivationFunctionType

N_CORES = 8
B, H, W, C = 32, 128, 128, 16
HID = 128
IMGS = B // N_CORES          # 4 images per core
GRP = IMGS * 2               # 8 (img, half) groups per core
PW = W + 2                   # padded row pitch 130
PR = H // 2 + 2              # padded rows per half 66
NPAD = PR * PW               # 8580
NPIX = (H // 2) * W          # 8192 interior pixels per group
CHUNK = 1024                 # pixels per MLP chunk (8 interior rows)
NCHUNK = NPIX // CHUNK       # 8
X2G = 128                    # guard elems around x2 free dim
RELU_PATTERN = (True, True, True, False)  # True -> ScalarE
# strip-in-half t -> partition block position: quarters {1-4},{5-8},{9-12},
# {13,14,15,0} unlock after chunks 2/4/6/7 respectively
TORD = [1, 2, 3, 4, 5, 6, 7, 8, 9, 10, 11, 12, 13, 14, 15, 0]
POS = {t: i for i, t in enumerate(TORD)}


def _split_multiwaits(nc):
    """walrus in this env only supports one sem-wait per instruction."""
    n = 0
    for f in nc.m.functions:
        for bb in f.blocks:
            out = []
            changed = False
            for inst in bb.instructions:
                si = inst.sync_info
                if si is not None and len(si.on_wait) > 1:
                    waits = list(si.on_wait)
                    for k, w in enumerate(waits[:-1]):
                        nop = mybir.InstNoOp(
                            name=f"{inst.name}_ws{k}",
                            sync_info=mybir.SyncInfo(on_wait=[w], on_update=[]),
                            bass_nofuse=True,
                            engine=inst.engine,
                        )
                        nc.register_instruction(nop, overwrite=True)
                        out.append(nop)
                        n += 1
                    inst.sync_info = mybir.SyncInfo(
                        on_wait=[waits[-1]], on_update=list(si.on_update)
                    )
                    changed = True
                out.append(inst)
            if changed:
                bb.instructions[:] = out
    return n


def _mk_ap(ap, offset, dims):
    return dataclasses.replace(ap, offset=offset, ap=[list(d) for d in dims])


def build_program():
    nc = bass.Bass()

    xb_d = nc.dram_tensor("xb", [128, NPAD], BF16, kind="ExternalInput").ap()
    x8_d = nc.dram_tensor("x8", [128, NPIX], FP8, kind="ExternalInput").ap()
    u16_d = nc.dram_tensor("u16", [128, NPIX], FP8, kind="ExternalInput").ap()
    astrip_d = nc.dram_tensor("astrip", [128, 780], F32, kind="ExternalInput").ap()
    # fp8 DoubleRow stationaries: pair A = (W0dx, W0id), pair B = (0, W0dy)
    w0_d = {}
    for pair in ("a", "b"):
        for gg in range(2):
            for j in range(4):
                w0_d[(pair, gg, j)] = nc.dram_tensor(
                    f"w0{pair}{gg}{j}", [128, 256], FP8, kind="ExternalInput"
                ).ap()
    w1_d = [
        nc.dram_tensor(f"w1dr{j}", [128, 256], FP8, kind="ExternalInput").ap()
        for j in range(4)
    ]
    sel_d = nc.dram_tensor("sel", [128, 2048], BF16, kind="ExternalInput").ap()
    out_d = nc.dram_tensor("out", [128, NPIX], BF16, kind="ExternalOutput").ap()

    with tile.TileContext(nc) as tc:
        with tc.tile_pool(name="persist", bufs=1) as pp:
            # --- persistent tiles ---
            xb = pp.tile([128, NPAD + 4], BF16, tag="xb")        # data at +2
            # dense fp8 feature buffer: [ydx | x | ydy], each [128, NPIX]
            feat8 = pp.tile([128, 3 * NPIX], FP8, tag="feat8")
            astrip = pp.tile([128, 780], F32, tag="astrip")
            a2strip = pp.tile([128, 780], BF16, tag="a2strip")
            selt = pp.tile([128, 2048], BF16, tag="selt")
            w0t = {k: pp.tile([128, 256], FP8, tag=f"w0{k[0]}{k[1]}{k[2]}", name=f"w0t{k[0]}{k[1]}{k[2]}") for k in w0_d}
            w1t = [
                pp.tile([128, 256], FP8, tag=f"w1dr{j}", name=f"w1t{j}")
                for j in range(4)
            ]

            # --- input DMAs: warm-up/first-chunk weights + xb first (they
            # gate the PE warm-up burst and the conv), late-use tensors
            # (astrip/sel/u16) on the scalar HWDGE ring ---
            for k in (("a", 0, 0), ("b", 0, 0), ("a", 1, 0), ("b", 1, 0)):
                nc.sync.dma_start(out=w0t[k][:, :], in_=w0_d[k])
            CAST_BANDS = [(0, 18), (18, 34), (34, 50), (50, 66)]
            for lo, hi in CAST_BANDS:
                nc.sync.dma_start(
                    out=xb[:, 2 + lo * PW : 2 + hi * PW],
                    in_=xb_d[:, lo * PW : hi * PW],
                )
            nc.sync.dma_start(out=feat8[:, NPIX : 2 * NPIX], in_=x8_d)
            for k in w0_d:
                if k[2] != 0:
                    nc.sync.dma_start(out=w0t[k][:, :], in_=w0_d[k])
            for j in range(4):
                nc.sync.dma_start(out=w1t[j][:, :], in_=w1_d[j])
            nc.scalar.dma_start(out=astrip[:, :], in_=astrip_d)
            nc.scalar.dma_start(out=selt[:, :], in_=sel_d)

            # --- conv stages: stage s covers image rows [16s, 16s+16).
            # All temps stage-local (margin rows recomputed), so stages can
            # be emitted interleaved with the MLP chunks that consume them.
            P8 = 3 * NPIX        # feat8 partition pitch
            SROWS = 8            # image rows per conv stage
            PXX = (SROWS + 2) * PW      # xx2/t_b/th stage-tile pitch
            PTV = SROWS * PW + 4        # t_a/tv stage-tile pitch

            def emit_stage(s):
                pr0 = SROWS * s              # th/xx2 base padded row
                n_th = min(pr0 + SROWS + 2, 66) - pr0
                il = pr0 + 1                 # interior padded row range
                xx2s = pp.tile([128, PXX], BF16, tag="xx2", bufs=2)
                t_as = pp.tile([128, PTV], BF16, tag="ta", bufs=2)
                tvs = pp.tile([128, PTV], BF16, tag="tv", bufs=2)
                t_bs = pp.tile([128, PXX + 4], BF16, tag="tb", bufs=2)
                ths = pp.tile([128, PXX + 4], BF16, tag="th", bufs=2)
                nc.scalar.activation(
                    out=xx2s[:, 0 : n_th * PW],
                    in_=xb[:, 2 + pr0 * PW : 2 + (pr0 + n_th) * PW],
                    func=ACTF.Copy, scale=2.0,
                )
                nc.vector.tensor_tensor(
                    out=t_bs[:, 2 : 2 + n_th * PW],
                    in0=xb[:, 1 + pr0 * PW : 1 + (pr0 + n_th) * PW],
                    in1=xb[:, 3 + pr0 * PW : 3 + (pr0 + n_th) * PW],
                    op=ALU.add,
                )
                nc.vector.tensor_tensor(
                    out=ths[:, 2 : 2 + n_th * PW],
                    in0=t_bs[:, 2 : 2 + n_th * PW],
                    in1=xx2s[:, 0 : n_th * PW],
                    op=ALU.add,
                )
                nc.vector.tensor_tensor(
                    out=t_as[:, 2 : 2 + SROWS * PW],
                    in0=xb[:, 2 + (il - 1) * PW : 2 + (il + SROWS - 1) * PW],
                    in1=xb[:, 2 + (il + 1) * PW : 2 + (il + SROWS + 1) * PW],
                    op=ALU.add,
                )
                nc.vector.tensor_tensor(
                    out=tvs[:, 2 : 2 + SROWS * PW],
                    in0=t_as[:, 2 : 2 + SROWS * PW],
                    in1=xx2s[:, 1 * PW : (SROWS + 1) * PW],
                    op=ALU.add,
                )
                # ydx -> feat8[0], ydy -> feat8[2], dense rows [8s,8s+8)
                nc.vector.tensor_tensor(
                    out=_mk_ap(feat8[:, :], SROWS * s * 128,
                               [[P8, 128], [128, SROWS], [1, 128]]),
                    in0=_mk_ap(tvs[:, :], 4, [[PTV, 128], [PW, SROWS], [1, 128]]),
                    in1=_mk_ap(tvs[:, :], 2, [[PTV, 128], [PW, SROWS], [1, 128]]),
                    op=ALU.subtract,
                )
                nc.vector.tensor_tensor(
                    out=_mk_ap(feat8[:, :], 2 * NPIX + SROWS * s * 128,
                               [[P8, 128], [128, SROWS], [1, 128]]),
                    in0=_mk_ap(ths[:, :], 2 * PW + 3,
                               [[PXX + 4, 128], [PW, SROWS], [1, 128]]),
                    in1=_mk_ap(ths[:, :], 3,
                               [[PXX + 4, 128], [PW, SROWS], [1, 128]]),
                    op=ALU.subtract,
                )
            emit_stage(0)
            emit_stage(1)

            # --- MLP + residual ---
            xbr = xb[:, 2 : 2 + NPAD].rearrange("p (r w) -> p r w", w=PW)
            xintr = xbr

            relu_i = 0
            lp_cm = tc.tile_pool(name="late", bufs=1)
            lp = lp_cm.__enter__()
            x2 = lp.tile([128, NPIX + 2 * X2G], BF16, tag="x2")  # data at +X2G
            nc.vector.memset(x2[:, 0:X2G], 0.0)
            nc.vector.memset(x2[:, X2G + NPIX : NPIX + 2 * X2G], 0.0)
            u16 = lp.tile([128, NPIX], FP8, tag="ul", name="u16")
            nc.scalar.dma_start(out=u16[:, :], in_=u16_d)
            x2r = x2[:, X2G : X2G + NPIX].rearrange("p (r w) -> p r w", w=W)
            with (
                tc.tile_pool(name="mlp", bufs=1) as mp,
                tc.tile_pool(name="psum", bufs=1, space="PSUM") as psp,
            ):
                prepool = pp.tile([128, 512], F32, tag="prepool")

                # dummy matmul burst: >=3.4us of sustained PE activity lifts
                # the HAM clock gate to 2.4 GHz before the real MLP begins
                warm = psp.tile([128, 512], F32, tag="lps", bufs=2)
                for _w in range(32):
                    nc.tensor.matmul(
                        warm[:, 0:256],
                        w0t[("a", 0, 0)][:, 0:128],
                        w0t[("b", 0, 0)][:, :],
                        start=True, stop=True,
                    )

                def emit_prepool():
                    vm_e = pp.tile([128, 524], F32, tag="vm_e")
                    t1_e = pp.tile([128, 524], F32, tag="t1_e")
                    t2_e = pp.tile([128, 524], F32, tag="t2_e")
                    nc.vector.tensor_tensor(
                        out=t1_e[:, 0:520], in0=astrip[:, 0:520],
                        in1=astrip[:, 130:650], op=ALU.max,
                    )
                    nc.vector.tensor_tensor(
                        out=vm_e[:, 0:520], in0=t1_e[:, 0:520],
                        in1=astrip[:, 260:780], op=ALU.max,
                    )
                    nc.vector.tensor_tensor(
                        out=t2_e[:, 0:519], in0=vm_e[:, 0:519], in1=vm_e[:, 1:520],
                        op=ALU.max,
                    )
                    _vmr = vm_e[:, 0:520].rearrange("p (r w) -> p r w", w=130)
                    _t2r = t2_e[:, 0:520].rearrange("p (r w) -> p r w", w=130)
                    _ppr = prepool[:, :].rearrange("p (r w) -> p r w", w=128)
                    nc.vector.tensor_tensor(
                        out=_ppr[:, 0:4, :], in0=_t2r[:, 0:4, 0:128],
                        in1=_vmr[:, 0:4, 2:130], op=ALU.max,
                    )

                # --- per-quarter alive-mask machinery ---
                PITCH = NPIX + 2 * X2G
                x2ap = x2[:, :]
                a2ap = a2strip[:, :]
                nc.vector.memset(a2strip[:, :], 0.0)
                postpool = lp.tile([128, 512], F32, tag="postpool")
                pmin = lp.tile([128, 512], F32, tag="pmin")
                lifes = lp.tile([128, 512], BF16, tag="lifes")

                def scatter_t(t):
                    # halo rows for strip t, all 8 (img,half) groups
                    src = _mk_ap(
                        x2ap, 3 * PITCH + 512 * t,
                        [[16 * PITCH, 8], [128, 6], [1, 128]],
                    )
                    dst = _mk_ap(
                        a2ap, 780 * 8 * POS[t] + 1,
                        [[780, 8], [130, 6], [1, 128]],
                    )
                    nc.sync.dma_start(out=dst, in_=src)

                def pool_range(pl, ph):
                    sl = slice(pl, ph)
                    vm = lp.tile([128, 524], F32, tag="vm")
                    t1 = lp.tile([128, 524], F32, tag="t1")
                    t2 = lp.tile([128, 524], F32, tag="t2")
                    nc.vector.tensor_tensor(
                        out=t1[sl, 0:520], in0=a2strip[sl, 0:520],
                        in1=a2strip[sl, 130:650], op=ALU.max,
                    )
                    nc.vector.tensor_tensor(
                        out=vm[sl, 0:520], in0=t1[sl, 0:520],
                        in1=a2strip[sl, 260:780], op=ALU.max,
                    )
                    nc.vector.tensor_tensor(
                        out=t2[sl, 0:519], in0=vm[sl, 0:519], in1=vm[sl, 1:520],
                        op=ALU.max,
                    )
                    vmr = vm[sl, 0:520].rearrange("p (r w) -> p r w", w=130)
                    t2r = t2[sl, 0:520].rearrange("p (r w) -> p r w", w=130)
                    ppr = postpool[sl, :].rearrange("p (r w) -> p r w", w=128)
                    nc.vector.tensor_tensor(
                        out=ppr[:, 0:4, :], in0=t2r[:, 0:4, 0:128],
                        in1=vmr[:, 0:4, 2:130], op=ALU.max,
                    )
                    nc.vector.tensor_tensor(
                        out=pmin[sl, :], in0=prepool[sl, :],
                        in1=postpool[sl, :], op=ALU.min,
                    )
                    nc.vector.tensor_scalar(
                        out=lifes[sl, :], in0=pmin[sl, :], scalar1=0.1,
                        scalar2=None, op0=ALU.is_gt,
                    )

                def finish_t(t):
                    p0 = 32 * (POS[t] // 4)
                    lps = psp.tile([128, 512], F32, tag="lps", bufs=2)
                    nc.tensor.matmul(
                        lps[:, :],
                        selt[p0 : p0 + 32, 128 * t : 128 * t + 128],
                        lifes[p0 : p0 + 32, 0:512],
                        start=True, stop=True,
                        tile_position=(p0, 0),
                    )
                    ot = lp.tile([128, 512], BF16, tag="ot", bufs=2)
                    nc.vector.tensor_tensor(
                        out=ot[:, :],
                        in0=x2[:, X2G + 512 * t : X2G + 512 * t + 512],
                        in1=lps[:, :], op=ALU.mult,
                    )
                    eng = nc.sync if t % 2 == 0 else nc.scalar
                    eng.dma_start(
                        out=out_d[:, 512 * t : 512 * t + 512], in_=ot[:, :]
                    )

                for k in range(NCHUNK):
                    if k < NCHUNK - 2:
                        emit_stage(k + 2)
                    if k == 1:
                        emit_prepool()
                    if k == 3:
                        for t in (0, 1, 2, 3, 4):
                            scatter_t(t)
                        pool_range(0, 32)
                        for t in (1, 2, 3, 4):
                            finish_t(t)
                    if k == 5:
                        for t in (5, 6, 7, 8):
                            scatter_t(t)
                        pool_range(32, 64)
                        for t in (5, 6, 7, 8):
                            finish_t(t)
                    if k == 7:
                        for t in (9, 10, 11, 12):
                            scatter_t(t)
                        pool_range(64, 96)
                        for t in (9, 10, 11, 12):
                            finish_t(t)
                    r0 = 8 * k  # interior row base of chunk
                    P8 = 3 * NPIX
                    dxp = [
                        psp.tile([128, 512], F32, tag="dxp", bufs=2,
                                 name=f"dxp{k}_{_s}")
                        for _s in range(2)
                    ]
                    for j in range(4):
                        rh2 = mp.tile([128, 2048], FP8, tag="rh", bufs=4)
                        for gg in range(2):
                            hp = psp.tile([128, CHUNK], F32, tag="hp", bufs=2)
                            for sub in range(2):
                                rr = r0 + 4 * sub
                                hps = hp[:, sub * 512 : sub * 512 + 512]
                                # k-tile pairs: A = (ydx, x), B = (x*0, ydy)
                                rhsA = _mk_ap(
                                    feat8[:, :], rr * 128,
                                    [[P8, 128], [NPIX, 2], [1, 512]],
                                )
                                rhsB = _mk_ap(
                                    feat8[:, :], NPIX + rr * 128,
                                    [[P8, 128], [NPIX, 2], [1, 512]],
                                )
                                lhsA = _mk_ap(
                                    w0t[("a", gg, j)][:, :], 0,
                                    [[256, 128], [128, 2], [1, 128]],
                                )
                                lhsB = _mk_ap(
                                    w0t[("b", gg, j)][:, :], 0,
                                    [[256, 128], [128, 2], [1, 128]],
                                )
                                nc.tensor.matmul(
                                    hps, lhsA, rhsA,
                                    start=True, stop=False, perf_mode=DR,
                                )
                                nc.tensor.matmul(
                                    hps, lhsB, rhsB,
                                    start=False, stop=True, perf_mode=DR,
                                )
                            rhs = rh2[:, gg * 1024 : gg * 1024 + 1024]
                            if relu_i < 16 or RELU_PATTERN[relu_i % len(RELU_PATTERN)]:
                                nc.scalar.activation(
                                    out=rhs, in_=hp[:, :], func=ACTF.Relu
                                )
                            else:
                                nc.vector.tensor_scalar_max(rhs, hp[:, :], 0.0)
                            relu_i += 1
                        for sub in range(2):
                            rhs1 = _mk_ap(
                                rh2[:, :], sub * 512,
                                [[2048, 128], [1024, 2], [1, 512]],
                            )
                            lhs1 = _mk_ap(
                                w1t[j][:, :], 0, [[256, 128], [128, 2], [1, 128]],
                            )
                            nc.tensor.matmul(
                                dxp[sub][:, :],
                                lhs1, rhs1,
                                start=(j == 0), stop=(j == 3), perf_mode=DR,
                            )
                    st = lp.tile([128, 2048], BF16, tag="st", name=f"st{k}", bufs=2)
                    for sub in range(2):
                        nc.vector.tensor_tensor(
                            out=st[:, sub * 512 : sub * 512 + 512],
                            in0=dxp[sub][:, :],
                            in1=u16[:, k * CHUNK + sub * 512 :
                                    k * CHUNK + sub * 512 + 512],
                            op=ALU.mult,
                        )
                    str_ = st[:, 0:CHUNK].rearrange("p (r w) -> p r w", w=W)
                    nc.gpsimd.tensor_tensor(
                        out=x2r[:, r0 : r0 + 8, :],
                        in0=xintr[:, 1 + r0 : 9 + r0, 1:129],
                        in1=str_,
                        op=ALU.add,
                    )

                # --- last quarter: strips 13,14,15,0 (need final x2) ---
                for t in (13, 14, 15):
                    scatter_t(t)
                # cross-half halo fills: h0/t15 row5 <- half1 row 0;
                # h1/t0 row0 <- half0 row 63
                nc.sync.dma_start(
                    out=_mk_ap(a2ap, (8 * POS[15]) * 780 + 5 * 130 + 1,
                               [[780 * 2, 4], [1, 128]]),
                    in_=_mk_ap(x2ap, 19 * PITCH + X2G,
                               [[32 * PITCH, 4], [1, 128]]),
                )
                nc.sync.dma_start(
                    out=_mk_ap(a2ap, (8 * POS[0] + 1) * 780 + 1,
                               [[780 * 2, 4], [1, 128]]),
                    in_=_mk_ap(x2ap, 3 * PITCH + X2G + 63 * 128,
                               [[32 * PITCH, 4], [1, 128]]),
                )
                pool_range(96, 128)
                for t in (13, 14, 15, 0):
                    finish_t(t)

            lp_cm.__exit__(None, None, None)

    _split_multiwaits(nc)
    return nc


def host_prep(x, w0, w1, rand_mask):
    bf = ml_dtypes.bfloat16
    f8 = ml_dtypes.float8_e4m3fn
    S = 8.0
    xt = np.ascontiguousarray(x.transpose(0, 3, 1, 2))  # [B, C, H, W]

    xp = np.zeros((B, 2, C, PR, PW), bf)
    xp[:, 0, :, 1:66, 1:129] = xt[:, :, 0:65, :].astype(bf)
    xp[:, 1, :, 0:65, 1:129] = xt[:, :, 63:128, :].astype(bf)
    xp = xp.reshape(B, 2, C, NPAD)

    x8 = np.stack([xt[:, :, 0:64, :], xt[:, :, 64:128, :]], axis=1)  # [B,2,C,64,W]
    x8 = x8.astype(f8).reshape(B, 2, C, NPIX)

    # dxp comes out scaled by S*S (weights pre-scaled for fp8) -> fold 1/S^2
    u = (rand_mask[..., 0] <= 0.5).astype(np.float32).reshape(B, 2, 64, W) / (S * S)
    u16 = np.ascontiguousarray(
        np.broadcast_to(u[:, :, None], (B, 2, C, 64, W))
    ).astype(f8).reshape(B, 2, C, NPIX)

    apad = np.zeros((B, H + 2, PW), np.float32)
    apad[:, 1:129, 1:129] = x[..., 3]
    idx = 4 * np.arange(32)[:, None] + np.arange(6)[None, :]
    astr = apad[:, idx, :].reshape(B, 32, 780)  # [B, strip, 6*130]
    # partition layout (pos, i, h): p = 8*POS[t] + 2i + h  (strip s = 16h + t)
    astr = astr.reshape(B // IMGS, IMGS, 2, 16, 780).transpose(0, 3, 1, 2, 4)
    astr = astr[:, TORD]

    # fp8 weights, pre-scaled by S (the sobel /8 cancels S for dx/dy)
    blk_id = (w0[:, 0::3] * S).T.astype(f8)   # [16 c, 128 o]
    blk_dx = w0[:, 1::3].T.astype(f8)
    blk_dy = w0[:, 2::3].T.astype(f8)
    w0_arrs = {}
    for gg in range(2):
        for j in range(4):
            ta = np.zeros((128, 2, 128), f8)
            tb = np.zeros((128, 2, 128), f8)
            r = slice(32 * j + 16 * gg, 32 * j + 16 * gg + 16)
            ta[r, 0, :] = blk_dx     # k-tile 0 pairs with ydx
            ta[r, 1, :] = blk_id     # k-tile 1 pairs with x
            tb[r, 1, :] = blk_dy     # k-tile 0 is x * 0, tile 1 is ydy
            w0_arrs[("a", gg, j)] = ta.reshape(128, 256)
            w0_arrs[("b", gg, j)] = tb.reshape(128, 256)
    w1_arrs = []
    for j in range(4):
        t = np.zeros((128, 2, 128), f8)
        t[:, 0, 32 * j : 32 * j + 16] = (w1.T * S).astype(f8)
        t[:, 1, 32 * j + 16 : 32 * j + 32] = (w1.T * S).astype(f8)
        w1_arrs.append(t.reshape(128, 256))

    sel = np.zeros((128, 2048), bf)
    for t in range(16):
        for p in range(128):
            g = p // 16  # g = 2*i + h for p = 32i + 16h + c
            sel[8 * POS[t] + g, 128 * t + p] = 1.0

    in_maps = []
    for ci in range(N_CORES):
        sl = slice(IMGS * ci, IMGS * (ci + 1))
        m = {
            "xb": np.ascontiguousarray(xp[sl]).reshape(128, NPAD),
            "x8": np.ascontiguousarray(x8[sl]).reshape(128, NPIX),
            "u16": np.ascontiguousarray(u16[sl]).reshape(128, NPIX),
            "astrip": np.ascontiguousarray(astr[ci]).reshape(128, 780),
            "sel": sel,
        }
        for j in range(4):
            m[f"w1dr{j}"] = w1_arrs[j]
        for (pair, gg, j), arr in w0_arrs.items():
            m[f"w0{pair}{gg}{j}"] = arr
        in_maps.append(m)
    return in_maps


def host_post(results):
    out = np.empty((B, H, W, C), np.float32)
    for ci in range(N_CORES):
        o = results[ci]["out"].reshape(IMGS, 2, C, 64, W)
        out[IMGS * ci : IMGS * (ci + 1)] = o.transpose(0, 1, 3, 4, 2).reshape(
            IMGS, H, W, C
        )
    return out


_CACHE = {}


def kernel(x, w0, w1, rand_mask, _trace=False, _tmpdir=None):
    x = np.asarray(x, np.float32)
    w0 = np.asarray(w0, np.float32)
    w1 = np.asarray(w1, np.float32)
    rand_mask = np.asarray(rand_mask, np.float32)

    if "nc" not in _CACHE:
        _CACHE["nc"] = build_program()
    nc = _CACHE["nc"]

    in_maps = host_prep(x, w0, w1, rand_mask)
    res = bass_utils.run_bass_kernel_spmd(
        nc, in_maps, core_ids=list(range(N_CORES)), trace=_trace, tmpdir=_tmpdir
    )
    _CACHE["last_result"] = res
    return host_post(res.results)

